# revision 1
# baseline (speedup 1.0000x reference)
"""CrossAttention kernel for 8x TRN2 NeuronCores (Bass/Tile).

Reference computation (per batch b of 16, heads h=8, n=1024, d_model=512, dh=64):
    q = x @ Wq.T, k = x @ Wk.T, v = x @ Wv.T          (per-head slices)
    sim = q k^T * scale + rel_bias[h]
    attn = softmax(sim, axis=-1)
    out = (attn @ v) re-assembled over heads, then @ Wo.T + bo

Sharding: data-parallel over batch, 2 batches per core x 8 cores.

Per-core device algorithm (all matmuls run as float32r = full-rate fp32-ish):
  - host supplies x^T per batch ([512, 1024]) and pre-transposed weights, so
    every matmul's contraction dim lands on SBUF partitions with no on-device
    transposes.
  - sim is computed TRANSPOSED: simT[j, i] = (K Q^T)[j, i], j on partitions.
    Softmax max-subtraction is skipped (logits are O(1) by construction);
    exp runs on ACT, the rel_bias add is folded as exp(sim)*exp(bias) with
    exp(bias^T) precomputed on host (bf16), multiplied on DVE in bf16 (2x mode).
  - attn @ V becomes OT[d, i] = sum_j Vaug[j, d|1] * et[j, i] on the PE with V
    in its NATURAL layout as lhsT; an appended ones-column yields the softmax
    denominator l[i] in the same matmul. Normalization multiplies by 1/l
    broadcast across partitions via a K=1 matmul.
  - final projection consumes attn_out^T directly as lhsT; bo is added via a
    K=1 ones matmul into the same PSUM accumulation group.
"""

import numpy as np
import ml_dtypes

HEADS = 8
DH = 64
B = 16
N = 1024
D = 512  # d_model = inner
SCALE = DH ** -0.5
NCORES = 8
BPC = B // NCORES  # batches per core

F32 = None  # set lazily (mybir import)


def build_nc(n=N, bpc=BPC, ps_a_bufs=3, ot_bufs=4, eb_bufs=6, eqk_bufs=6, et_bufs=15, xt_bufs=None, phases="PAF", debug=False):
    import concourse.mybir as mybir
    import concourse.tile as tile
    from concourse import bacc

    f32 = mybir.dt.float32
    f32r = mybir.dt.float32r
    bf16 = mybir.dt.bfloat16
    Exp = mybir.ActivationFunctionType.Exp

    NT = n // 128            # n tiles of 128
    NIH = max(1, n // 512)   # i-halves
    IW = min(512, n)         # i-slice width (psum free dim)
    KP = D // 128            # d_model k-tiles (4)
    HP = HEADS // 2          # head pairs

    nc = bacc.Bacc(None, target_bir_lowering=False)

    xT_d = nc.dram_tensor("xT", [bpc, D, n], f32r, kind="ExternalInput")
    wq_d = nc.dram_tensor("WqT", [D, D], f32r, kind="ExternalInput")   # pre-scaled
    wk_d = nc.dram_tensor("WkT", [D, D], f32r, kind="ExternalInput")
    wv_d = nc.dram_tensor("WvT", [D, D], f32r, kind="ExternalInput")
    wo_d = nc.dram_tensor("WoT", [D, D], f32r, kind="ExternalInput")
    eb_d = nc.dram_tensor("expBT", [HEADS, n, n], bf16, kind="ExternalInput")
    bo_d = nc.dram_tensor("bo", [1, D], f32r, kind="ExternalInput")
    ones_d = nc.dram_tensor("ones", [1, 128], f32r, kind="ExternalInput")
    out_d = nc.dram_tensor("out", [bpc, n, D], f32, kind="ExternalOutput")
    if debug:
        dqt_d = nc.dram_tensor("dQT", [128, n], f32, kind="ExternalOutput")
        dkt_d = nc.dram_tensor("dKT", [128, n], f32, kind="ExternalOutput")
        dva_d = nc.dram_tensor("dVA", [128, HEADS * (DH + 1)], mybir.dt.bfloat16, kind="ExternalOutput")
        det_d = nc.dram_tensor("dET", [128, n], mybir.dt.bfloat16, kind="ExternalOutput")
        dot_d = nc.dram_tensor("dOT", [DH + 1, 512], f32, kind="ExternalOutput")
        dao_d = nc.dram_tensor("dAO", [128, n], f32, kind="ExternalOutput")

    with tile.TileContext(nc) as tc:
        with (
            tc.tile_pool(name="pers", bufs=1) as pers,       # persistent tiles
            tc.tile_pool(name="osb", bufs=4) as osbp,
            tc.tile_pool(name="lr", bufs=2) as lrp,
        ):
            # ---- persistent tiles
            QT = [[pers.tile([128, n], f32r, tag=f"qt{bi}_{ip}", name=f"qt{bi}_{ip}") for ip in range(KP)]
                  for bi in range(bpc)]
            KT = [[pers.tile([128, n], f32r, tag=f"kt{bi}_{ip}", name=f"kt{bi}_{ip}") for ip in range(KP)]
                  for bi in range(bpc)]
            VA = [[pers.tile([128, HEADS * (DH + 1)], bf16, tag=f"va{bi}_{nt}", name=f"va{bi}_{nt}")
                   for nt in range(NT)] for bi in range(bpc)]
            AO = [[pers.tile([128, n], f32r, tag=f"ao{bi}_{kp}", name=f"ao{bi}_{kp}") for kp in range(KP)]
                  for bi in range(bpc)]
            wo_s = [pers.tile([128, D], f32r, tag=f"wo{kp}", name=f"wo{kp}") for kp in range(KP)]
            bo_s = pers.tile([1, D], f32r, tag="bo")
            ones128 = pers.tile([1, 128], f32r, tag="ones128")
            nc.sync.dma_start(out=ones128[:], in_=ones_d[:])
            nc.sync.dma_start(out=bo_s[:], in_=bo_d[:])
            for kp in range(KP):
                nc.sync.dma_start(out=wo_s[kp][:], in_=wo_d[kp * 128:(kp + 1) * 128, :])

            wqkv = tc.alloc_tile_pool(name="wqkv", bufs=1)
            xtp = tc.alloc_tile_pool(name="xt", bufs=xt_bufs or (KP + 1))
            ps_p = tc.alloc_tile_pool(name="ps_p", bufs=4, space="PSUM")
            wq_s = [wqkv.tile([128, D], f32r, tag=f"wq{kp}", name=f"wq{kp}") for kp in range(KP)]
            wk_s = [wqkv.tile([128, D], f32r, tag=f"wk{kp}", name=f"wk{kp}") for kp in range(KP)]
            wv_s = [wqkv.tile([128, D], f32r, tag=f"wv{kp}", name=f"wv{kp}") for kp in range(KP)]
            for kp in range(KP):
                nc.sync.dma_start(out=wq_s[kp][:], in_=wq_d[kp * 128:(kp + 1) * 128, :])
                nc.sync.dma_start(out=wk_s[kp][:], in_=wk_d[kp * 128:(kp + 1) * 128, :])
                nc.sync.dma_start(out=wv_s[kp][:], in_=wv_d[kp * 128:(kp + 1) * 128, :])

            # ================= Phase P: projections =================
            for bi in (range(bpc) if "P" in phases else ()):
                xt = [xtp.tile([128, n], f32r, tag="xt", name="xt") for _ in range(KP)]
                for kp in range(KP):
                    nc.sync.dma_start(out=xt[kp][:], in_=xT_d[bi, kp * 128:(kp + 1) * 128, :])
                # QT/KT: [inner, n] = W^T.T @ x^T
                for W_s, DST in ((wq_s, QT[bi]), (wk_s, KT[bi])):
                    for ip in range(KP):
                        for nh in range(NIH):
                            pt = ps_p.tile([128, IW], f32, tag="mm")
                            for kp in range(KP):
                                nc.tensor.matmul(
                                    pt[:],
                                    W_s[kp][:, ip * 128:(ip + 1) * 128],
                                    xt[kp][:, nh * IW:(nh + 1) * IW],
                                    start=(kp == 0), stop=(kp == KP - 1),
                                )
                            nc.vector.tensor_copy(
                                out=DST[ip][:, nh * IW:(nh + 1) * IW], in_=pt[:])
                # V natural [n, inner] -> VA bf16 with ones cols
                for nt in range(NT):
                    pt = ps_p.tile([128, D], f32, tag="mm")
                    for kp in range(KP):
                        nc.tensor.matmul(
                            pt[:],
                            xt[kp][:, nt * 128:(nt + 1) * 128],
                            wv_s[kp][:],
                            start=(kp == 0), stop=(kp == KP - 1),
                        )
                    va = VA[bi][nt]
                    nc.gpsimd.memset(va[:], 1.0)
                    dst3 = va[:].rearrange("p (h c) -> p h c", c=DH + 1)[:, :, 0:DH]
                    src3 = pt[:].rearrange("p (h c) -> p h c", c=DH)
                    nc.vector.tensor_copy(out=dst3, in_=src3)

            ps_p.release()
            xtp.release()
            wqkv.release()
            ebp = tc.alloc_tile_pool(name="eb", bufs=eb_bufs)
            eqkp = tc.alloc_tile_pool(name="eqk", bufs=eqk_bufs)
            etp = tc.alloc_tile_pool(name="et", bufs=et_bufs)
            ps_sim = tc.alloc_tile_pool(name="ps_sim", bufs=2, space="PSUM")
            ps_ot = tc.alloc_tile_pool(name="ps_ot", bufs=ot_bufs, space="PSUM")

            # ================= Phase A: attention =================
            for hp in (range(HP) if "A" in phases else ()):
                h0, h1 = 2 * hp, 2 * hp + 1
                streams = [(h, bi) for bi in range(bpc) for h in (h0, h1)]
                et_t = {}
                for jt in range(NT):
                    eb = {}
                    for h in (h0, h1):
                        t = ebp.tile([128, n], bf16, tag="eb", name="eb")
                        nc.sync.dma_start(out=t[:], in_=eb_d[h, jt * 128:(jt + 1) * 128, :])
                        eb[h] = t
                    for (h, bi) in streams:
                        po = (h % 2) * 64
                        ktile = KT[bi][h // 2]
                        qtile = QT[bi][h // 2]
                        sp = ps_sim.tile([128, n], f32, tag="sim", name="sim")
                        for ihh in range(NIH):
                            nc.tensor.matmul(
                                sp[:, ihh * IW:(ihh + 1) * IW],
                                ktile[po:po + 64, jt * 128:(jt + 1) * 128],
                                qtile[po:po + 64, ihh * IW:(ihh + 1) * IW],
                                start=True, stop=True,
                            )
                        eq = eqkp.tile([128, n], bf16, tag="eqk")
                        nc.scalar.activation(eq[:], sp[:], Exp)
                        et = etp.tile([128, n], bf16, tag="et")
                        nc.vector.tensor_mul(out=et[:], in0=eq[:], in1=eb[h][:])
                        et_t[(h, bi, jt)] = et
                        if debug and h == 0 and bi == 0 and jt == 0:
                            nc.sync.dma_start(out=det_d[:], in_=et[:])
                for (h, bi) in streams:
                    po = (h % 2) * 64
                    for ihh in range(NIH):
                        o = ps_ot.tile([DH + 1, IW], f32, tag="ot", name="ot")
                        for jt in range(NT):
                            nc.tensor.matmul(
                                o[:],
                                VA[bi][jt][:, h * (DH + 1):(h + 1) * (DH + 1)],
                                et_t[(h, bi, jt)][:, ihh * IW:(ihh + 1) * IW],
                                start=(jt == 0), stop=(jt == NT - 1),
                            )
                        if debug and h == 0 and bi == 0 and ihh == 0:
                            ots = osbp.tile([DH + 1, IW], f32, tag="dots", name="dots", bufs=1)
                            nc.vector.tensor_copy(out=ots[:], in_=o[:])
                            nc.sync.dma_start(out=dot_d[:], in_=ots[:])
                        lcp = lrp.tile([DH + 1, IW], f32, tag="lcp", name="lcp")
                        nc.vector.tensor_copy(out=lcp[DH:DH + 1, :], in_=o[DH:DH + 1, :])
                        lr0 = lrp.tile([1, IW], f32, tag="lr0", name="lr0")
                        nc.sync.dma_start(out=lr0[:], in_=lcp[DH:DH + 1, :])
                        lrr = lrp.tile([1, IW], f32, tag="lrr", name="lrr")
                        nc.vector.reciprocal_approx_fast(out=lrr[:], in_=lr0[:])
                        lb = osbp.tile([DH, IW], f32, tag="lb", name="lb", bufs=2)
                        nc.gpsimd.partition_broadcast(lb[:], lrr[:], channels=DH)
                        if po == 0:
                            nc.vector.tensor_mul(
                                out=AO[bi][h // 2][0:DH, ihh * IW:(ihh + 1) * IW],
                                in0=o[0:DH, :], in1=lb[:])
                        else:
                            tmpo = osbp.tile([DH, IW], f32r, tag="tmpo", name="tmpo", bufs=2)
                            nc.vector.tensor_mul(out=tmpo[:], in0=o[0:DH, :], in1=lb[:])
                            nc.sync.dma_start(
                                out=AO[bi][h // 2][po:po + DH, ihh * IW:(ihh + 1) * IW],
                                in_=tmpo[:])

            ps_ot.release()
            ps_sim.release()
            etp.release()
            eqkp.release()
            ebp.release()
            ps_f = tc.alloc_tile_pool(name="ps_f", bufs=4, space="PSUM")

            # ================= Phase F: output projection =================
            for bi in (range(bpc) if "F" in phases else ()):
                for nt in range(NT):
                    fp = ps_f.tile([128, D], f32, tag="mm")
                    for kp in range(KP):
                        nc.tensor.matmul(
                            fp[:],
                            AO[bi][kp][:, nt * 128:(nt + 1) * 128],
                            wo_s[kp][:],
                            start=(kp == 0), stop=False,
                        )
                    nc.tensor.matmul(fp[:], ones128[:], bo_s[:],
                                     start=False, stop=True)
                    fo = osbp.tile([128, D], f32, tag="fo", name="fo")
                    nc.vector.tensor_copy(out=fo[:], in_=fp[:])
                    nc.sync.dma_start(out=out_d[bi, nt * 128:(nt + 1) * 128, :], in_=fo[:])
            ps_f.release()
            if debug:
                nc.sync.dma_start(out=dqt_d[:], in_=QT[0][0][:].bitcast(f32))
                nc.sync.dma_start(out=dkt_d[:], in_=KT[0][0][:].bitcast(f32))
                nc.sync.dma_start(out=dva_d[:], in_=VA[0][0][:])
                nc.sync.dma_start(out=dao_d[:], in_=AO[0][0][:].bitcast(f32))

    nc.compile()
    return nc


def prep_inputs(x, Wq, Wk, Wv, rel_bias, Wo, bo, n=N, bpc=BPC, ncores=NCORES):
    """Host-side sharding/layout prep. Returns in_maps (one dict per core)."""
    x = np.ascontiguousarray(x, dtype=np.float32)
    xT = np.ascontiguousarray(x.transpose(0, 2, 1))        # [B, D, n]
    WqT = np.ascontiguousarray(Wq.T * np.float32(SCALE), dtype=np.float32)
    WkT = np.ascontiguousarray(Wk.T, dtype=np.float32)
    WvT = np.ascontiguousarray(Wv.T, dtype=np.float32)
    WoT = np.ascontiguousarray(Wo.T, dtype=np.float32)
    expBT = np.ascontiguousarray(
        np.exp(rel_bias.astype(np.float32).transpose(0, 2, 1))
    ).astype(ml_dtypes.bfloat16)                            # [H, n(j), n(i)]
    bo2 = np.ascontiguousarray(bo, dtype=np.float32).reshape(1, D)
    in_maps = []
    for c in range(ncores):
        in_maps.append({
            "xT": np.ascontiguousarray(xT[c * bpc:(c + 1) * bpc]),
            "WqT": WqT, "WkT": WkT, "WvT": WvT, "WoT": WoT,
            "expBT": expBT, "bo": bo2, "ones": np.ones((1, 128), np.float32),
        })
    return in_maps


_CACHE = {}


def kernel(x, Wq, Wk, Wv, rel_bias, Wo, bo):
    from concourse.bass_utils import run_bass_kernel_spmd

    if "nc" not in _CACHE:
        _CACHE["nc"] = build_nc()
    nc = _CACHE["nc"]
    in_maps = prep_inputs(x, Wq, Wk, Wv, rel_bias, Wo, bo)
    res = run_bass_kernel_spmd(nc, in_maps, core_ids=list(range(NCORES)))
    out = np.concatenate([res.results[c]["out"] for c in range(NCORES)], axis=0)
    return np.ascontiguousarray(out, dtype=np.float32)



# revision 11
# speedup vs baseline: 1.0871x; 1.0871x over previous
"""CrossAttention kernel for 8x TRN2 NeuronCores (Bass/Tile), v2.

Reference computation (per batch b of 16, heads h=8, n=1024, d_model=512, dh=64):
    q = x @ Wq.T, k = x @ Wk.T, v = x @ Wv.T          (per-head slices)
    sim = q k^T * scale + rel_bias[h]
    attn = softmax(sim, axis=-1)
    out = (attn @ v) re-assembled over heads, then @ Wo.T + bo

Sharding: data-parallel over batch, 2 batches per core x 8 cores.

v2 design notes (vs v1 baseline at ~264us modeled):
  - all matmul operands fp16 (full-rate on PE at any tile size, better
    mantissa than bf16, halves weight/x/eb DMA vs f32).
  - softmax runs on transposed sim (j on partitions); rel_bias applied as
    exp(sim)*exp(bias^T) with exp(bias^T) precomputed on host in fp16;
    the multiply runs in-place on DVE in 16-bit 2x mode.
  - attn@V uses V in natural layout as lhsT with an appended ones column:
    the same matmul emits the softmax denominator l as psum row 64.
  - normalization: reciprocal straight off the psum l-row into row 64 of
    the broadcast tile (DVE), gpsimd partition_broadcast from partition 64,
    one 1024-wide DVE multiply. Odd heads land in AO rows 64..127 via one
    SBUF->SBUF DMA.
  - output bias bo is added on the host after the gather.
  - PSUM split into two rings: "mm" (sim tiles only, so ACT never starves
    behind injected work) and "ot" (attn@V accumulators + all projection /
    output-projection chunks).
  - coarse DMAs: one descriptor-chain per weight matrix / x batch, rel-bias
    loaded in jt-pairs - halves serial HWDGE occupancy.
  - emission is software-pipelined: projections for ip0 first, then the
    attention stream loop with V/QK(ip1..3)/F chunks injected between
    sim slots so the PE never starves while ACT grinds exp.
"""

import numpy as np
import ml_dtypes

HEADS = 8
DH = 64
B = 16
N = 1024
D = 512  # d_model = inner
SCALE = DH ** -0.5
NCORES = 8
BPC = B // NCORES  # batches per core


def build_nc(n=N, bpc=BPC, sim_bufs=2, ot_bufs=2, eb_bufs=7, et_bufs=18,
             lb_bufs=2, fo_bufs=4, tmpo_bufs=2, inject_stride=2):
    import concourse.mybir as mybir
    import concourse.tile as tile
    from concourse import bacc

    f32 = mybir.dt.float32
    f16 = mybir.dt.float16
    Exp = mybir.ActivationFunctionType.Exp
    Copy = mybir.ActivationFunctionType.Copy

    NT = n // 128            # n tiles of 128 (8)
    KP = D // 128            # d_model k-tiles (4)
    HP = HEADS // 2          # head pairs (4)
    T2 = NT // 2             # double-nt chunks (4)

    nc = bacc.Bacc(None, target_bir_lowering=False)

    xT_d = nc.dram_tensor("xT", [bpc, D, n], f16, kind="ExternalInput")
    wq_d = nc.dram_tensor("WqT", [D, D], f16, kind="ExternalInput")   # pre-scaled
    wk_d = nc.dram_tensor("WkT", [D, D], f16, kind="ExternalInput")
    wv_d = nc.dram_tensor("WvT", [D, D], f16, kind="ExternalInput")
    wo_d = nc.dram_tensor("WoT", [D, D], f16, kind="ExternalInput")
    eb_d = nc.dram_tensor("expBT", [HEADS, n, n], f16, kind="ExternalInput")
    out_d = nc.dram_tensor("out", [bpc, n, D], f32, kind="ExternalOutput")

    with tile.TileContext(nc) as tc:
        pers = tc.alloc_tile_pool(name="pers", bufs=1)
        # ---- persistent tiles
        QT = [[pers.tile([128, n], f16, tag=f"qt{bi}_{ip}", name=f"qt{bi}_{ip}")
               for ip in range(KP)] for bi in range(bpc)]
        KT = [[pers.tile([128, n], f16, tag=f"kt{bi}_{ip}", name=f"kt{bi}_{ip}")
               for ip in range(KP)] for bi in range(bpc)]
        VA = [[pers.tile([128, HEADS * (DH + 1)], f16, tag=f"va{bi}_{nt}",
                         name=f"va{bi}_{nt}") for nt in range(NT)]
              for bi in range(bpc)]
        AO = [[pers.tile([128, n], f16, tag=f"ao{bi}_{kp}", name=f"ao{bi}_{kp}")
               for kp in range(KP)] for bi in range(bpc)]
        # each weight matrix lives in one [128, KP, 512] tile (one DMA each)
        w4 = {}
        for wname, wd in (("q", wq_d), ("k", wk_d), ("v", wv_d), ("o", wo_d)):
            t = pers.tile([128, KP, D], f16, tag=f"w{wname}", name=f"w{wname}")
            w4[wname] = t
        xtp = tc.alloc_tile_pool(name="xt", bufs=1)
        xt = [xtp.tile([128, KP, n], f16, tag=f"x{bi}", name=f"x{bi}")
              for bi in range(bpc)]

        ebp = tc.alloc_tile_pool(name="eb", bufs=eb_bufs)
        etp = tc.alloc_tile_pool(name="et", bufs=et_bufs)
        lrp = tc.alloc_tile_pool(name="lr", bufs=lb_bufs)
        fop = tc.alloc_tile_pool(name="fop", bufs=1)
        ps = tc.alloc_tile_pool(name="ps", bufs=1, space="PSUM")

        # ---- prologue DMAs (one chained DMA per tensor)
        def dma_w(wname, wd):
            dst = w4[wname][:]
            src = wd.rearrange("(kp p) c -> p kp c", p=128)
            nc.sync.dma_start(out=dst, in_=src)

        dma_w("q", wq_d)
        for bi in range(bpc):
            nc.sync.dma_start(
                out=xt[bi][:], in_=xT_d[bi].rearrange("(kp p) j -> p kp j", p=128))
        dma_w("k", wk_d)
        dma_w("v", wv_d)
        dma_w("o", wo_d)

        eb_tiles = {}

        def emit_eb_dma(h, jp):
            """Load jt pair (2*jp, 2*jp+1) of head h as one [128, 2, n] tile."""
            t = ebp.tile([128, 2, n], f16, tag="eb", name="eb")
            nc.sync.dma_start(
                out=t[:],
                in_=eb_d[h, 2 * jp * 128:(2 * jp + 2) * 128, :].rearrange(
                    "(two p) i -> p two i", two=2))
            eb_tiles[(h, 2 * jp)] = t[:, 0, :]
            eb_tiles[(h, 2 * jp + 1)] = t[:, 1, :]

        for jp in range(NT // 2):
            emit_eb_dma(0, jp)

        # ---- emitters -----------------------------------------------------
        def emit_qk_chunk(wname, DST, bi, ip):
            W_s = w4[wname]
            pt = ps.tile([128, 1024], f32, tag="ot", bufs=ot_bufs, name="pt")
            for nh in range(2):
                for kp in range(KP):
                    nc.tensor.matmul(
                        pt[:, nh * 512:(nh + 1) * 512],
                        W_s[:, kp, ip * 128:(ip + 1) * 128],
                        xt[bi][:, kp, nh * 512:(nh + 1) * 512],
                        start=(kp == 0), stop=(kp == KP - 1),
                    )
            nc.vector.tensor_copy(out=DST[bi][ip][:], in_=pt[:])

        def emit_v_chunk(bi, t2):
            pt = ps.tile([128, 1024], f32, tag="ot", bufs=ot_bufs, name="pt")
            for b in range(2):
                nt = 2 * t2 + b
                for kp in range(KP):
                    nc.tensor.matmul(
                        pt[:, b * 512:(b + 1) * 512],
                        xt[bi][:, kp, nt * 128:(nt + 1) * 128],
                        w4["v"][:, kp, :],
                        start=(kp == 0), stop=(kp == KP - 1),
                    )
            for b in range(2):
                nt = 2 * t2 + b
                va = VA[bi][nt]
                nc.gpsimd.memset(va[:], 1.0)
                dst3 = va[:].rearrange("p (h c) -> p h c", c=DH + 1)[:, :, 0:DH]
                src3 = pt[:, b * 512:(b + 1) * 512].rearrange("p (h c) -> p h c", c=DH)
                nc.vector.tensor_copy(out=dst3, in_=src3)

        def emit_f_chunk(bi, t2):
            pt = ps.tile([128, 1024], f32, tag="ot", bufs=ot_bufs, name="pt")
            for b in range(2):
                nt = 2 * t2 + b
                for kp in range(KP):
                    nc.tensor.matmul(
                        pt[:, b * 512:(b + 1) * 512],
                        AO[bi][kp][:, nt * 128:(nt + 1) * 128],
                        w4["o"][:, kp, :],
                        start=(kp == 0), stop=(kp == KP - 1),
                    )
            for b in range(2):
                nt = 2 * t2 + b
                fo = fop.tile([128, 512], f32, tag="fo", bufs=fo_bufs, name="fo")
                nc.scalar.activation(fo[:], pt[:, b * 512:(b + 1) * 512], Copy)
                nc.sync.dma_start(out=out_d[bi, nt * 128:(nt + 1) * 128, :], in_=fo[:])

        def emit_av(h, bi, et_row):
            ot = ps.tile([DH + 1, 1024], f32, tag="ot", bufs=ot_bufs, name="ot")
            for ihh in range(2):
                for jt in range(NT):
                    nc.tensor.matmul(
                        ot[:, ihh * 512:(ihh + 1) * 512],
                        VA[bi][jt][:, h * (DH + 1):(h + 1) * (DH + 1)],
                        et_row[jt][:, ihh * 512:(ihh + 1) * 512],
                        start=(jt == 0), stop=(jt == NT - 1),
                    )
            # baseline-proven norm chain: copy psum l-row to SBUF, DMA it to
            # partition 0, reciprocal there, broadcast from partition 0.
            lb = lrp.tile([DH + 1, n], f32, tag="lb", name="lb")
            nc.vector.tensor_copy(out=lb[DH:DH + 1, :], in_=ot[DH:DH + 1, :])
            lr0 = lrp.tile([1, n], f32, tag="lr0", name="lr0")
            nc.sync.dma_start(out=lr0[:], in_=lb[DH:DH + 1, :])
            lrr = lrp.tile([1, n], f32, tag="lrr", name="lrr")
            nc.vector.reciprocal_approx_fast(out=lrr[:], in_=lr0[:])
            nc.gpsimd.partition_broadcast(lb[0:DH, :], lrr[:], channels=DH)
            if h % 2 == 0:
                nc.vector.tensor_mul(
                    out=AO[bi][h // 2][0:DH, :], in0=ot[0:DH, :], in1=lb[0:DH, :])
            else:
                tmpo = lrp.tile([DH, n], f16, tag="tmpo", bufs=tmpo_bufs, name="tmpo")
                nc.vector.tensor_mul(out=tmpo[:], in0=ot[0:DH, :], in1=lb[0:DH, :])
                nc.sync.dma_start(out=AO[bi][h // 2][DH:128, :], in_=tmpo[:])

        # ---- prologue compute: QK projections for ip=0 (heads 0,1)
        for bi in range(bpc):
            emit_qk_chunk("q", QT, bi, 0)
            emit_qk_chunk("k", KT, bi, 0)

        # ---- inject queue: V, then QK ip1..3, F appended later
        inject = []
        for bi in range(bpc):
            for t2 in range(T2):
                inject.append(("v", bi, t2))
        for ip in range(1, KP):
            for bi in range(bpc):
                inject.append(("q", bi, ip))
                inject.append(("k", bi, ip))

        def pop_inject():
            if not inject:
                return
            kind, a0, a1 = inject.pop(0)
            if kind == "v":
                emit_v_chunk(a0, a1)
            elif kind == "q":
                emit_qk_chunk("q", QT, a0, a1)
            elif kind == "k":
                emit_qk_chunk("k", KT, a0, a1)
            elif kind == "f":
                emit_f_chunk(a0, a1)

        # ---- main attention loop
        slot = 0
        for hp in range(HP):
            h0, h1 = 2 * hp, 2 * hp + 1
            streams = [(h0, 0), (h0, 1), (h1, 0), (h1, 1)]
            for sidx, (h, bi) in enumerate(streams):
                et_row = []
                for jt in range(NT):
                    # eb prefetch (jt pairs): sidx1 fetches h1's tiles,
                    # sidx2 fetches the next head-pair's h0 tiles.
                    if jt % 2 == 0:
                        if sidx == 1:
                            emit_eb_dma(h1, jt // 2)
                        elif sidx == 2 and hp + 1 < HP:
                            emit_eb_dma(2 * (hp + 1), jt // 2)
                    sp = ps.tile([128, 1024], f32, tag="mm", bufs=sim_bufs, name="sp")
                    po = (h % 2) * 64
                    for ihh in range(2):
                        nc.tensor.matmul(
                            sp[:, ihh * 512:(ihh + 1) * 512],
                            KT[bi][hp][po:po + 64, jt * 128:(jt + 1) * 128],
                            QT[bi][hp][po:po + 64, ihh * 512:(ihh + 1) * 512],
                            start=True, stop=True,
                        )
                    eq = etp.tile([128, n], f16, tag="eq", bufs=3, name="eq")
                    nc.scalar.activation(eq[:], sp[:], Exp)
                    et = etp.tile([128, n], f16, tag="et", name="et")
                    nc.vector.tensor_mul(out=et[:], in0=eq[:], in1=eb_tiles[(h, jt)])
                    et_row.append(et)
                    if slot % inject_stride == 0:
                        pop_inject()
                    slot += 1
                emit_av(h, bi, et_row)
                # after the last b0 stream of the last head pair, queue F(b0)
                if hp == HP - 1 and (h, bi) == (h1, 0):
                    for t2 in range(T2):
                        inject.append(("f", 0, t2))
        # drain: F for batch 1 (+ anything left)
        while inject:
            pop_inject()
        for t2 in range(T2):
            emit_f_chunk(1, t2)

        for p in (ps, fop, lrp, etp, ebp, xtp, pers):
            p.release()

    nc.compile()
    return nc


def prep_inputs(x, Wq, Wk, Wv, rel_bias, Wo, bo, n=N, bpc=BPC, ncores=NCORES):
    """Host-side sharding/layout prep. Returns in_maps (one dict per core)."""
    f16 = np.float16
    x = np.asarray(x, dtype=np.float32)
    xT = np.ascontiguousarray(x.transpose(0, 2, 1)).astype(f16)   # [B, D, n]
    WqT = np.ascontiguousarray(Wq.T * np.float32(SCALE)).astype(f16)
    WkT = np.ascontiguousarray(Wk.T).astype(f16)
    WvT = np.ascontiguousarray(Wv.T).astype(f16)
    WoT = np.ascontiguousarray(Wo.T).astype(f16)
    expBT = np.ascontiguousarray(
        np.exp(np.asarray(rel_bias, dtype=np.float32).transpose(0, 2, 1))
    ).astype(f16)                                                  # [H, n(j), n(i)]
    in_maps = []
    for c in range(ncores):
        in_maps.append({
            "xT": np.ascontiguousarray(xT[c * bpc:(c + 1) * bpc]),
            "WqT": WqT, "WkT": WkT, "WvT": WvT, "WoT": WoT,
            "expBT": expBT,
        })
    return in_maps


_CACHE = {}


def kernel(x, Wq, Wk, Wv, rel_bias, Wo, bo):
    from concourse.bass_utils import run_bass_kernel_spmd

    if "nc" not in _CACHE:
        _CACHE["nc"] = build_nc()
    nc = _CACHE["nc"]
    in_maps = prep_inputs(x, Wq, Wk, Wv, rel_bias, Wo, bo)
    res = run_bass_kernel_spmd(nc, in_maps, core_ids=list(range(NCORES)))
    out = np.concatenate([res.results[c]["out"] for c in range(NCORES)], axis=0)
    out = out + np.asarray(bo, dtype=np.float32)[None, None, :]
    return np.ascontiguousarray(out, dtype=np.float32)


# revision 31
# speedup vs baseline: 1.2059x; 1.1093x over previous
"""CrossAttention kernel for 8x TRN2 NeuronCores (Bass/Tile), v2.

Reference computation (per batch b of 16, heads h=8, n=1024, d_model=512, dh=64):
    q = x @ Wq.T, k = x @ Wk.T, v = x @ Wv.T          (per-head slices)
    sim = q k^T * scale + rel_bias[h]
    attn = softmax(sim, axis=-1)
    out = (attn @ v) re-assembled over heads, then @ Wo.T + bo

Sharding: data-parallel over batch, 2 batches per core x 8 cores.

v2 design notes (vs v1 baseline at ~264us modeled):
  - all matmul operands fp16 (full-rate on PE at any tile size, better
    mantissa than bf16, halves weight/x/eb DMA vs f32).
  - softmax runs on transposed sim (j on partitions); rel_bias applied as
    exp(sim)*exp(bias^T) with exp(bias^T) precomputed on host in fp16;
    the multiply runs in-place on DVE in 16-bit 2x mode.
  - attn@V uses V in natural layout as lhsT with an appended ones column:
    the same matmul emits the softmax denominator l as psum row 64.
  - normalization (HW constraint: custom DVE ops and partition_broadcast
    only operate from partition 0, and only on SBUF): DVE-copy the psum
    l-row to SBUF, DMA it to partition 0, reciprocal_approx_fast there,
    gpsimd partition_broadcast, one 1024-wide DVE multiply. Odd heads land
    in AO rows 64..127 via one SBUF->SBUF DMA.
  - output is fp16 (host converts to f32 and adds bias bo) - halves the
    serial output-DMA drain at the end of the program.
  - PSUM split into two rings: "mm" (sim tiles only, so ACT never starves
    behind injected work) and "ot" (attn@V accumulators + all projection /
    output-projection chunks).
  - coarse DMAs: one descriptor-chain per weight matrix / x batch, rel-bias
    loaded in jt-pairs - halves serial HWDGE occupancy.
  - emission is software-pipelined: projections for ip0 first, then the
    attention stream loop with V/QK(ip1..3)/F chunks injected between
    sim slots so the PE never starves while ACT grinds exp.
"""

import numpy as np

HEADS = 8
DH = 64
B = 16
N = 1024
D = 512  # d_model = inner
SCALE = DH ** -0.5
NCORES = 8
BPC = B // NCORES  # batches per core


def build_nc(n=N, bpc=BPC, sim_bufs=2, ot_bufs=2, eb_bufs=7, et_bufs=18,
             lb_bufs=2, fo_bufs=4, tmpo_bufs=2, pool_mul_mod=0):
    import concourse.mybir as mybir
    import concourse.tile as tile
    from concourse import bacc

    f32 = mybir.dt.float32
    f16 = mybir.dt.float16
    Exp = mybir.ActivationFunctionType.Exp
    Copy = mybir.ActivationFunctionType.Copy

    NT = n // 128            # n tiles of 128 (8)
    KP = D // 128            # d_model k-tiles (4)
    HP = HEADS // 2          # head pairs (4)
    T2 = NT // 2             # double-nt chunks (4)

    nc = bacc.Bacc(None, target_bir_lowering=False)

    xT_d = nc.dram_tensor("xT", [bpc, D, n], f16, kind="ExternalInput")
    wq_d = nc.dram_tensor("WqT", [D, D], f16, kind="ExternalInput")   # pre-scaled
    wk_d = nc.dram_tensor("WkT", [D, D], f16, kind="ExternalInput")
    wv_d = nc.dram_tensor("WvT", [D, D], f16, kind="ExternalInput")
    wo_d = nc.dram_tensor("WoT", [D, D], f16, kind="ExternalInput")
    eb_d = nc.dram_tensor("expBT", [HEADS, n, n], f16, kind="ExternalInput")
    out_d = nc.dram_tensor("out", [bpc, n, D], f16, kind="ExternalOutput")

    with tile.TileContext(nc) as tc:
        pers = tc.alloc_tile_pool(name="pers", bufs=1)
        # ---- persistent tiles
        QT = [[pers.tile([128, n], f16, tag=f"qt{bi}_{ip}", name=f"qt{bi}_{ip}")
               for ip in range(KP)] for bi in range(bpc)]
        KT = [[pers.tile([128, n], f16, tag=f"kt{bi}_{ip}", name=f"kt{bi}_{ip}")
               for ip in range(KP)] for bi in range(bpc)]
        VA = [[pers.tile([128, HEADS * (DH + 1)], f16, tag=f"va{bi}_{nt}",
                         name=f"va{bi}_{nt}") for nt in range(NT)]
              for bi in range(bpc)]
        AO = [[pers.tile([128, n], f16, tag=f"ao{bi}_{kp}", name=f"ao{bi}_{kp}")
               for kp in range(KP)] for bi in range(bpc)]
        # each weight matrix lives in one [128, KP, 512] tile (one DMA each)
        w4 = {}
        for wname, wd in (("q", wq_d), ("k", wk_d), ("v", wv_d), ("o", wo_d)):
            t = pers.tile([128, KP, D], f16, tag=f"w{wname}", name=f"w{wname}")
            w4[wname] = t
        xtp = tc.alloc_tile_pool(name="xt", bufs=1)
        xt = [xtp.tile([128, KP, n], f16, tag=f"x{bi}", name=f"x{bi}")
              for bi in range(bpc)]

        ebp = tc.alloc_tile_pool(name="eb", bufs=eb_bufs)
        etp = tc.alloc_tile_pool(name="et", bufs=et_bufs)
        lrp = tc.alloc_tile_pool(name="lr", bufs=lb_bufs)
        fop = tc.alloc_tile_pool(name="fop", bufs=1)
        ps = tc.alloc_tile_pool(name="ps", bufs=1, space="PSUM")

        # ---- prologue DMAs (one chained DMA per tensor)
        def dma_w(wname, wd):
            dst = w4[wname][:]
            src = wd.rearrange("(kp p) c -> p kp c", p=128)
            nc.sync.dma_start(out=dst, in_=src)

        def dma_x(bi):
            nc.sync.dma_start(
                out=xt[bi][:], in_=xT_d[bi].rearrange("(kp p) j -> p kp j", p=128))

        pool_jts = set(range(3, 3 + pool_mul_mod))
        av_jt_order = [j for j in range(NT) if j not in pool_jts] + sorted(pool_jts)
        eb_tiles = {}

        def emit_eb_dma(h, jp):
            """Load jt pair (2*jp, 2*jp+1) of head h as one [128, 2, n] tile."""
            t = ebp.tile([128, 2, n], f16, tag="eb", name="eb")
            nc.sync.dma_start(
                out=t[:],
                in_=eb_d[h, 2 * jp * 128:(2 * jp + 2) * 128, :].rearrange(
                    "(two p) i -> p two i", two=2))
            eb_tiles[(h, 2 * jp)] = t[:, 0, :]
            eb_tiles[(h, 2 * jp + 1)] = t[:, 1, :]

        dma_w("q", wq_d)
        dma_x(0)
        dma_w("k", wk_d)
        dma_x(1)
        dma_w("v", wv_d)
        dma_w("o", wo_d)
        for jp in range(NT // 2):
            emit_eb_dma(0, jp)

        # ---- emitters -----------------------------------------------------
        inj_ring = [0]

        def inj_tag():
            return "ot"

        def emit_qk_chunk(wname, DST, bi, ip):
            W_s = w4[wname]
            pt = ps.tile([128, 1024], f32, tag=inj_tag(), bufs=ot_bufs, name="pt")
            for nh in range(2):
                for kp in range(KP):
                    nc.tensor.matmul(
                        pt[:, nh * 512:(nh + 1) * 512],
                        W_s[:, kp, ip * 128:(ip + 1) * 128],
                        xt[bi][:, kp, nh * 512:(nh + 1) * 512],
                        start=(kp == 0), stop=(kp == KP - 1),
                    )
            if ip <= qk_copy_act_ip:
                nc.scalar.activation(DST[bi][ip][:], pt[:], Copy)
            else:
                nc.vector.tensor_copy(out=DST[bi][ip][:], in_=pt[:])

        def emit_v_chunk(bi, t2):
            pt = ps.tile([128, 1024], f32, tag=inj_tag(), bufs=ot_bufs, name="pt")
            for b in range(2):
                nt = 2 * t2 + b
                for kp in range(KP):
                    nc.tensor.matmul(
                        pt[:, b * 512:(b + 1) * 512],
                        xt[bi][:, kp, nt * 128:(nt + 1) * 128],
                        w4["v"][:, kp, :],
                        start=(kp == 0), stop=(kp == KP - 1),
                    )
            for b in range(2):
                nt = 2 * t2 + b
                va = VA[bi][nt]
                nc.gpsimd.memset(va[:], 1.0)
                dst3 = va[:].rearrange("p (h c) -> p h c", c=DH + 1)[:, :, 0:DH]
                src3 = pt[:, b * 512:(b + 1) * 512].rearrange("p (h c) -> p h c", c=DH)
                nc.vector.tensor_copy(out=dst3, in_=src3)

        def emit_f_chunk(bi, t2):
            pt = ps.tile([128, 1024], f32, tag=inj_tag(), bufs=ot_bufs, name="pt")
            for b in range(2):
                nt = 2 * t2 + b
                for kp in range(KP):
                    nc.tensor.matmul(
                        pt[:, b * 512:(b + 1) * 512],
                        AO[bi][kp][:, nt * 128:(nt + 1) * 128],
                        w4["o"][:, kp, :],
                        start=(kp == 0), stop=(kp == KP - 1),
                    )
            for b in range(2):
                nt = 2 * t2 + b
                fo = fop.tile([128, 512], f16, tag="fo", bufs=fo_bufs, name="fo")
                nc.scalar.activation(fo[:], pt[:, b * 512:(b + 1) * 512], Copy)
                nc.sync.dma_start(out=out_d[bi, nt * 128:(nt + 1) * 128, :], in_=fo[:])

        pend = {"av": None}

        def start_av(h, bi, et_row):
            ot = ps.tile([DH + 1, 1024], f32, tag="ot", bufs=ot_bufs, name="ot")
            mms = [(ihh, jt) for ihh in range(2) for jt in av_jt_order]
            pend["av"] = {"h": h, "bi": bi, "et": et_row, "ot": ot, "mms": mms}

        def step_av(nmm):
            st = pend["av"]
            if st is None:
                return
            h, bi, et_row, ot = st["h"], st["bi"], st["et"], st["ot"]
            while nmm > 0 and st["mms"]:
                ihh, jt = st["mms"].pop(0)
                nc.tensor.matmul(
                    ot[:, ihh * 512:(ihh + 1) * 512],
                    VA[bi][jt][:, h * (DH + 1):(h + 1) * (DH + 1)],
                    et_row[jt][:, ihh * 512:(ihh + 1) * 512],
                    start=(jt == av_jt_order[0]), stop=(jt == av_jt_order[-1]),
                )
                nmm -= 1
            if not st["mms"]:
                emit_norm(h, bi, ot)
                pend["av"] = None

        def emit_norm(h, bi, ot):
            # reciprocal straight off the psum l-row into row 64 of the
            # broadcast tile, gpsimd broadcast from partition 64.
            lb = lrp.tile([DH + 1, n], f32, tag="lb", name="lb")
            nc.vector.reciprocal_approx_fast(out=lb[DH:DH + 1, :], in_=ot[DH:DH + 1, :])
            nc.gpsimd.partition_broadcast(lb[0:DH, :], lb[DH:DH + 1, :], channels=DH)
            if h % 2 == 0:
                nc.vector.tensor_mul(
                    out=AO[bi][h // 2][0:DH, :], in0=ot[0:DH, :], in1=lb[0:DH, :])
            else:
                tmpo = lrp.tile([DH, n], f16, tag="tmpo", bufs=tmpo_bufs, name="tmpo")
                nc.vector.tensor_mul(out=tmpo[:], in0=ot[0:DH, :], in1=lb[0:DH, :])
                nc.sync.dma_start(out=AO[bi][h // 2][DH:128, :], in_=tmpo[:])

        # ---- prologue compute: QK projections for ip=0 (heads 0,1)
        for bi in range(bpc):
            emit_qk_chunk("q", QT, bi, 0)
            emit_qk_chunk("k", KT, bi, 0)

        # ---- deadline-scheduled inject: chunk -> emission slot
        sched = {}
        slots_v0 = [0, 2, 4, 6]
        slots_v1 = [8, 10, 12, 14]
        slots_ip = {1: [18, 21, 24, 27], 2: [36, 44, 52, 60], 3: [68, 76, 84, 92]}
        for t2 in range(T2):
            sched[slots_v0[t2]] = ("v", 0, t2)
            sched[slots_v1[t2]] = ("v", 1, t2)
        for ip in range(1, KP):
            for i, (kind, bi) in enumerate((("q", 0), ("k", 0), ("q", 1), ("k", 1))):
                sched[slots_ip[ip][i]] = (kind, bi, ip)
        for t2 in range(T2):
            sched[120 + 2 * t2] = ("f", 0, t2)

        def emit_chunk(c):
            kind, a0, a1 = c
            if kind == "v":
                emit_v_chunk(a0, a1)
            elif kind == "q":
                emit_qk_chunk("q", QT, a0, a1)
            elif kind == "k":
                emit_qk_chunk("k", KT, a0, a1)
            elif kind == "f":
                emit_f_chunk(a0, a1)

        # ---- main attention loop; last head pair runs h-odd first so the
        # final stream is an even head (its norm-mul writes AO directly,
        # shortening the tail by one SBUF->SBUF DMA hop).
        stream_order = {}
        for hp in range(HP):
            h0, h1 = 2 * hp, 2 * hp + 1
            if hp == HP - 1:
                stream_order[hp] = [(h1, 0), (h1, 1), (h0, 0), (h0, 1)]
            else:
                stream_order[hp] = [(h0, 0), (h0, 1), (h1, 0), (h1, 1)]

        slot = 0
        for hp in range(HP):
            streams = stream_order[hp]
            first_h = streams[0][0]
            other_h = streams[2][0]
            for sidx, (h, bi) in enumerate(streams):
                is_last = (hp == HP - 1 and sidx == 3)
                ot_last = None
                if is_last:
                    ot_last = ps.tile([DH + 1, 1024], f32, tag="ot",
                                      bufs=ot_bufs, name="ot_last")
                et_row = []
                for jt in range(NT):
                    # eb prefetch (jt pairs): sidx1 fetches this pair's other
                    # head, sidx2 fetches the next head-pair's first head.
                    if jt % 2 == 0:
                        if sidx == 1:
                            emit_eb_dma(other_h, jt // 2)
                        elif sidx == 2 and hp + 1 < HP:
                            emit_eb_dma(stream_order[hp + 1][0][0], jt // 2)
                    sp = ps.tile([128, 1024], f32, tag="mm", bufs=sim_bufs, name="sp")
                    po = (h % 2) * 64
                    for ihh in range(2):
                        nc.tensor.matmul(
                            sp[:, ihh * 512:(ihh + 1) * 512],
                            KT[bi][hp][po:po + 64, jt * 128:(jt + 1) * 128],
                            QT[bi][hp][po:po + 64, ihh * 512:(ihh + 1) * 512],
                            start=True, stop=True,
                        )
                    eq = etp.tile([128, n], f16, tag="eq", bufs=8, name="eq")
                    nc.scalar.activation(eq[:], sp[:], Exp)
                    et = etp.tile([128, n], f16, tag="et", name="et")
                    use_pool = (not is_last) and jt in pool_jts
                    mul_eng = nc.gpsimd if use_pool else nc.vector
                    mul_eng.tensor_mul(out=et[:], in0=eq[:], in1=eb_tiles[(h, jt)])
                    et_row.append(et)
                    if is_last:
                        for ihh in range(2):
                            nc.tensor.matmul(
                                ot_last[:, ihh * 512:(ihh + 1) * 512],
                                VA[bi][jt][:, h * (DH + 1):(h + 1) * (DH + 1)],
                                et[:, ihh * 512:(ihh + 1) * 512],
                                start=(jt == 0), stop=(jt == NT - 1),
                            )
                    if jt == av_flush_slot:
                        step_av(99)
                    if slot in sched:
                        emit_chunk(sched.pop(slot))
                    slot += 1
                if is_last:
                    emit_norm(h, bi, ot_last)
                else:
                    emit_av(h, bi, et_row)
        step_av(99)
        # drain: anything left, then F for batch 1
        for s in sorted(sched):
            emit_chunk(sched[s])
        for t2 in range(T2):
            emit_f_chunk(1, t2)

        for p in (ps, fop, lrp, etp, ebp, xtp, pers):
            p.release()

    nc.compile()
    return nc


def prep_inputs(x, Wq, Wk, Wv, rel_bias, Wo, bo, n=N, bpc=BPC, ncores=NCORES):
    """Host-side sharding/layout prep. Returns in_maps (one dict per core)."""
    f16 = np.float16
    x = np.asarray(x, dtype=np.float32)
    xT = np.ascontiguousarray(x.transpose(0, 2, 1)).astype(f16)   # [B, D, n]
    WqT = np.ascontiguousarray(Wq.T * np.float32(SCALE)).astype(f16)
    WkT = np.ascontiguousarray(Wk.T).astype(f16)
    WvT = np.ascontiguousarray(Wv.T).astype(f16)
    WoT = np.ascontiguousarray(Wo.T).astype(f16)
    expBT = np.ascontiguousarray(
        np.exp(np.asarray(rel_bias, dtype=np.float32).transpose(0, 2, 1))
    ).astype(f16)                                                  # [H, n(j), n(i)]
    in_maps = []
    for c in range(ncores):
        in_maps.append({
            "xT": np.ascontiguousarray(xT[c * bpc:(c + 1) * bpc]),
            "WqT": WqT, "WkT": WkT, "WvT": WvT, "WoT": WoT,
            "expBT": expBT,
        })
    return in_maps


_CACHE = {}


def kernel(x, Wq, Wk, Wv, rel_bias, Wo, bo):
    from concourse.bass_utils import run_bass_kernel_spmd

    if "nc" not in _CACHE:
        _CACHE["nc"] = build_nc()
    nc = _CACHE["nc"]
    in_maps = prep_inputs(x, Wq, Wk, Wv, rel_bias, Wo, bo)
    res = run_bass_kernel_spmd(nc, in_maps, core_ids=list(range(NCORES)))
    out = np.concatenate([res.results[c]["out"] for c in range(NCORES)], axis=0)
    out = out + np.asarray(bo, dtype=np.float32)[None, None, :]
    return np.ascontiguousarray(out, dtype=np.float32)


# revision 36
# speedup vs baseline: 1.2169x; 1.0091x over previous
"""CrossAttention kernel for 8x TRN2 NeuronCores (Bass/Tile), v2.

Reference computation (per batch b of 16, heads h=8, n=1024, d_model=512, dh=64):
    q = x @ Wq.T, k = x @ Wk.T, v = x @ Wv.T          (per-head slices)
    sim = q k^T * scale + rel_bias[h]
    attn = softmax(sim, axis=-1)
    out = (attn @ v) re-assembled over heads, then @ Wo.T + bo

Sharding: data-parallel over batch, 2 batches per core x 8 cores.

v2 design notes (vs v1 baseline at ~264us modeled):
  - all matmul operands fp16 (full-rate on PE at any tile size, better
    mantissa than bf16, halves weight/x/eb DMA vs f32).
  - softmax runs on transposed sim (j on partitions); rel_bias applied as
    exp(sim)*exp(bias^T) with exp(bias^T) precomputed on host in fp16;
    the multiply runs in-place on DVE in 16-bit 2x mode.
  - attn@V uses V in natural layout as lhsT with an appended ones column:
    the same matmul emits the softmax denominator l as psum row 64.
  - normalization (HW constraint: custom DVE ops and partition_broadcast
    only operate from partition 0, and only on SBUF): DVE-copy the psum
    l-row to SBUF, DMA it to partition 0, reciprocal_approx_fast there,
    gpsimd partition_broadcast, one 1024-wide DVE multiply. Odd heads land
    in AO rows 64..127 via one SBUF->SBUF DMA.
  - output is fp16 (host converts to f32 and adds bias bo) - halves the
    serial output-DMA drain at the end of the program.
  - PSUM split into two rings: "mm" (sim tiles only, so ACT never starves
    behind injected work) and "ot" (attn@V accumulators + all projection /
    output-projection chunks).
  - coarse DMAs: one descriptor-chain per weight matrix / x batch, rel-bias
    loaded in jt-pairs - halves serial HWDGE occupancy.
  - emission is software-pipelined: projections for ip0 first, then the
    attention stream loop with V/QK(ip1..3)/F chunks injected between
    sim slots so the PE never starves while ACT grinds exp.
"""

import numpy as np

HEADS = 8
DH = 64
B = 16
N = 1024
D = 512  # d_model = inner
SCALE = DH ** -0.5
NCORES = 8
BPC = B // NCORES  # batches per core


def build_nc(n=N, bpc=BPC, sim_bufs=2, ot_bufs=2, eb_bufs=7, et_bufs=18,
             lb_bufs=3, fo_bufs=6, tmpo_bufs=2, pool_mul_mod=0):
    import concourse.mybir as mybir
    import concourse.tile as tile
    from concourse import bacc

    f32 = mybir.dt.float32
    f16 = mybir.dt.float16
    Exp = mybir.ActivationFunctionType.Exp
    Copy = mybir.ActivationFunctionType.Copy

    NT = n // 128            # n tiles of 128 (8)
    KP = D // 128            # d_model k-tiles (4)
    HP = HEADS // 2          # head pairs (4)
    T2 = NT // 2             # double-nt chunks (4)

    nc = bacc.Bacc(None, target_bir_lowering=False)

    xT_d = nc.dram_tensor("xT", [bpc, D, n], f16, kind="ExternalInput")
    wq_d = nc.dram_tensor("WqT", [D, D], f16, kind="ExternalInput")   # pre-scaled
    wk_d = nc.dram_tensor("WkT", [D, D], f16, kind="ExternalInput")
    wv_d = nc.dram_tensor("WvT", [D, D], f16, kind="ExternalInput")
    wo_d = nc.dram_tensor("WoT", [D, D], f16, kind="ExternalInput")
    eb_d = nc.dram_tensor("expBT", [HEADS, n, n], f16, kind="ExternalInput")
    out_d = nc.dram_tensor("out", [bpc, n, D], f16, kind="ExternalOutput")

    with tile.TileContext(nc) as tc:
        pers = tc.alloc_tile_pool(name="pers", bufs=1)
        # ---- persistent tiles
        QT = [[pers.tile([128, n], f16, tag=f"qt{bi}_{ip}", name=f"qt{bi}_{ip}")
               for ip in range(KP)] for bi in range(bpc)]
        KT = [[pers.tile([128, n], f16, tag=f"kt{bi}_{ip}", name=f"kt{bi}_{ip}")
               for ip in range(KP)] for bi in range(bpc)]
        VA = [[pers.tile([128, HEADS * (DH + 1)], f16, tag=f"va{bi}_{nt}",
                         name=f"va{bi}_{nt}") for nt in range(NT)]
              for bi in range(bpc)]
        AO = [[pers.tile([128, n], f16, tag=f"ao{bi}_{kp}", name=f"ao{bi}_{kp}")
               for kp in range(KP)] for bi in range(bpc)]
        # each weight matrix lives in one [128, KP, 512] tile (one DMA each)
        w4 = {}
        for wname, wd in (("q", wq_d), ("k", wk_d), ("v", wv_d), ("o", wo_d)):
            t = pers.tile([128, KP, D], f16, tag=f"w{wname}", name=f"w{wname}")
            w4[wname] = t
        xtp = tc.alloc_tile_pool(name="xt", bufs=1)
        xt = [xtp.tile([128, KP, n], f16, tag=f"x{bi}", name=f"x{bi}")
              for bi in range(bpc)]

        ebp = tc.alloc_tile_pool(name="eb", bufs=eb_bufs)
        etp = tc.alloc_tile_pool(name="et", bufs=et_bufs)
        lrp = tc.alloc_tile_pool(name="lr", bufs=lb_bufs)
        fop = tc.alloc_tile_pool(name="fop", bufs=1)
        ps = tc.alloc_tile_pool(name="ps", bufs=1, space="PSUM")

        # ---- prologue DMAs (one chained DMA per tensor)
        def dma_w(wname, wd):
            dst = w4[wname][:]
            src = wd.rearrange("(kp p) c -> p kp c", p=128)
            nc.sync.dma_start(out=dst, in_=src)

        def dma_x(bi, split=False):
            if split:
                for kp in range(KP):
                    nc.sync.dma_start(
                        out=xt[bi][:, kp, :],
                        in_=xT_d[bi, kp * 128:(kp + 1) * 128, :])
            else:
                nc.sync.dma_start(
                    out=xt[bi][:], in_=xT_d[bi].rearrange("(kp p) j -> p kp j", p=128))

        pool_jts = set(range(3, 3 + pool_mul_mod))
        av_jt_order = [j for j in range(NT) if j not in pool_jts] + sorted(pool_jts)
        eb_tiles = {}

        def emit_eb_dma(h, jp):
            """Load jt pair (2*jp, 2*jp+1) of head h as one [128, 2, n] tile."""
            t = ebp.tile([128, 2, n], f16, tag="eb", name="eb")
            nc.sync.dma_start(
                out=t[:],
                in_=eb_d[h, 2 * jp * 128:(2 * jp + 2) * 128, :].rearrange(
                    "(two p) i -> p two i", two=2))
            eb_tiles[(h, 2 * jp)] = t[:, 0, :]
            eb_tiles[(h, 2 * jp + 1)] = t[:, 1, :]

        dma_w("q", wq_d)
        dma_x(0, split=bool(x0_split))
        dma_w("k", wk_d)
        dma_x(1)
        dma_w("v", wv_d)
        dma_w("o", wo_d)
        for jp in range(NT // 2):
            emit_eb_dma(0, jp)

        # ---- PE warm-up: keep the tensor engine continuously busy through
        # the prologue DMAs so its p-state clock is fully ramped (213ns/row
        # instead of 427+) when the first real projection matmuls arrive.
        if warmup_mms:
            scr = pers.tile([128, 512], f16, tag="scr", name="scr")
            nc.gpsimd.memset(scr[:], 0.0)
            wps = ps.tile([128, 512], f32, tag="mm", bufs=sim_bufs, name="wps")
            for _ in range(warmup_mms):
                nc.tensor.matmul(wps[:], scr[:, 0:128], scr[:],
                                 start=True, stop=True)

        # ---- emitters -----------------------------------------------------
        inj_ring = [0]

        def inj_tag():
            return "ot"

        def emit_qk_chunk(wname, DST, bi, ip, ring=None):
            W_s = w4[wname]
            pt = ps.tile([128, 1024], f32, tag=ring or inj_tag(), bufs=ot_bufs,
                         name="pt")
            for nh in range(2):
                for kp in range(KP):
                    nc.tensor.matmul(
                        pt[:, nh * 512:(nh + 1) * 512],
                        W_s[:, kp, ip * 128:(ip + 1) * 128],
                        xt[bi][:, kp, nh * 512:(nh + 1) * 512],
                        start=(kp == 0), stop=(kp == KP - 1),
                    )
            if ip <= qk_copy_act_ip:
                nc.scalar.activation(DST[bi][ip][:], pt[:], Copy)
            else:
                nc.vector.tensor_copy(out=DST[bi][ip][:], in_=pt[:])

        def emit_v_chunk(bi, t2):
            pt = ps.tile([128, 1024], f32, tag=inj_tag(), bufs=ot_bufs, name="pt")
            for b in range(2):
                nt = 2 * t2 + b
                for kp in range(KP):
                    nc.tensor.matmul(
                        pt[:, b * 512:(b + 1) * 512],
                        xt[bi][:, kp, nt * 128:(nt + 1) * 128],
                        w4["v"][:, kp, :],
                        start=(kp == 0), stop=(kp == KP - 1),
                    )
            for b in range(2):
                nt = 2 * t2 + b
                va = VA[bi][nt]
                nc.gpsimd.memset(va[:], 1.0)
                dst3 = va[:].rearrange("p (h c) -> p h c", c=DH + 1)[:, :, 0:DH]
                src3 = pt[:, b * 512:(b + 1) * 512].rearrange("p (h c) -> p h c", c=DH)
                nc.vector.tensor_copy(out=dst3, in_=src3)

        def emit_f_chunk(bi, t2, ring=None):
            pt = ps.tile([128, 1024], f32, tag=ring or inj_tag(), bufs=ot_bufs,
                         name="pt")
            for b in range(2):
                nt = 2 * t2 + b
                for kp in range(KP):
                    nc.tensor.matmul(
                        pt[:, b * 512:(b + 1) * 512],
                        AO[bi][kp][:, nt * 128:(nt + 1) * 128],
                        w4["o"][:, kp, :],
                        start=(kp == 0), stop=(kp == KP - 1),
                    )
            for b in range(2):
                nt = 2 * t2 + b
                fo = fop.tile([128, 512], f16, tag="fo", bufs=fo_bufs, name="fo")
                nc.scalar.activation(fo[:], pt[:, b * 512:(b + 1) * 512], Copy)
                nc.sync.dma_start(out=out_d[bi, nt * 128:(nt + 1) * 128, :], in_=fo[:])

        pend = {"av": None}

        def start_av(h, bi, et_row):
            ot = ps.tile([DH + 1, 1024], f32, tag="ot", bufs=ot_bufs, name="ot")
            mms = [(ihh, jt) for ihh in range(2) for jt in av_jt_order]
            pend["av"] = {"h": h, "bi": bi, "et": et_row, "ot": ot, "mms": mms}

        def step_av(nmm):
            st = pend["av"]
            if st is None:
                return
            h, bi, et_row, ot = st["h"], st["bi"], st["et"], st["ot"]
            while nmm > 0 and st["mms"]:
                ihh, jt = st["mms"].pop(0)
                nc.tensor.matmul(
                    ot[:, ihh * 512:(ihh + 1) * 512],
                    VA[bi][jt][:, h * (DH + 1):(h + 1) * (DH + 1)],
                    et_row[jt][:, ihh * 512:(ihh + 1) * 512],
                    start=(jt == av_jt_order[0]), stop=(jt == av_jt_order[-1]),
                )
                nmm -= 1
            if not st["mms"]:
                emit_norm(h, bi, ot)
                pend["av"] = None

        def emit_norm(h, bi, ot):
            # reciprocal straight off the psum l-row into row 64 of the
            # broadcast tile, gpsimd broadcast from partition 64.
            lb = lrp.tile([DH + 1, n], f32, tag="lb", name="lb")
            nc.vector.reciprocal_approx_fast(out=lb[DH:DH + 1, :], in_=ot[DH:DH + 1, :])
            nc.gpsimd.partition_broadcast(lb[0:DH, :], lb[DH:DH + 1, :], channels=DH)
            if h % 2 == 0:
                nc.vector.tensor_mul(
                    out=AO[bi][h // 2][0:DH, :], in0=ot[0:DH, :], in1=lb[0:DH, :])
            else:
                tmpo = lrp.tile([DH, n], f16, tag="tmpo", bufs=tmpo_bufs, name="tmpo")
                nc.vector.tensor_mul(out=tmpo[:], in0=ot[0:DH, :], in1=lb[0:DH, :])
                nc.sync.dma_start(out=AO[bi][h // 2][DH:128, :], in_=tmpo[:])

        # ---- prologue compute: QK projections for ip=0 (heads 0,1).
        # The sim ring is empty this early, so borrow it: four chunks then
        # rotate through four psum slots instead of two.
        for bi in range(bpc):
            emit_qk_chunk("q", QT, bi, 0, ring="mm" if proq_mm else None)
            emit_qk_chunk("k", KT, bi, 0, ring="mm" if proq_mm else None)

        # ---- deadline-scheduled inject: chunk -> emission slot
        sched = {}
        slots_v0 = [0, 2, 4, 6]
        slots_v1 = [8, 10, 12, 14]
        slots_ip = {1: [18, 21, 24, 27], 2: [36, 44, 52, 60], 3: [68, 76, 84, 92]}
        for t2 in range(T2):
            sched[slots_v0[t2]] = ("v", 0, t2)
            sched[slots_v1[t2]] = ("v", 1, t2)
        for ip in range(1, KP):
            for i, (kind, bi) in enumerate((("q", 0), ("k", 0), ("q", 1), ("k", 1))):
                sched[slots_ip[ip][i]] = (kind, bi, ip)
        for t2 in range(T2):
            sched[120 + 2 * t2] = ("f", 0, t2)

        def emit_chunk(c):
            kind, a0, a1 = c
            if kind == "v":
                emit_v_chunk(a0, a1)
            elif kind == "q":
                emit_qk_chunk("q", QT, a0, a1)
            elif kind == "k":
                emit_qk_chunk("k", KT, a0, a1)
            elif kind == "f":
                emit_f_chunk(a0, a1)

        # ---- main attention loop; last head pair runs h-odd first so the
        # final stream is an even head (its norm-mul writes AO directly,
        # shortening the tail by one SBUF->SBUF DMA hop).
        stream_order = {}
        for hp in range(HP):
            h0, h1 = 2 * hp, 2 * hp + 1
            if hp == HP - 1:
                stream_order[hp] = [(h1, 0), (h1, 1), (h0, 0), (h0, 1)]
            else:
                stream_order[hp] = [(h0, 0), (h0, 1), (h1, 0), (h1, 1)]

        slot = 0
        for hp in range(HP):
            streams = stream_order[hp]
            first_h = streams[0][0]
            other_h = streams[2][0]
            for sidx, (h, bi) in enumerate(streams):
                is_last = (hp == HP - 1 and sidx == 3)
                ot_last = None
                if is_last:
                    ot_last = ps.tile([DH + 1, 1024], f32, tag="ot",
                                      bufs=ot_bufs, name="ot_last")
                et_row = []
                sp_hoist = [None]
                for jt in range(NT):
                    # eb prefetch (jt pairs): sidx1 fetches this pair's other
                    # head, sidx2 fetches the next head-pair's first head.
                    if jt % 2 == 0:
                        if sidx == 1:
                            emit_eb_dma(other_h, jt // 2)
                        elif sidx == 2 and hp + 1 < HP:
                            emit_eb_dma(stream_order[hp + 1][0][0], jt // 2)
                    po = (h % 2) * 64

                    def emit_sim(j):
                        spj = ps.tile([128, 1024], f32, tag="mm", bufs=sim_bufs,
                                      name="spj")
                        for ihh in range(2):
                            nc.tensor.matmul(
                                spj[:, ihh * 512:(ihh + 1) * 512],
                                KT[bi][hp][po:po + 64, j * 128:(j + 1) * 128],
                                QT[bi][hp][po:po + 64, ihh * 512:(ihh + 1) * 512],
                                start=True, stop=True,
                            )
                        return spj

                    if jt == 1 and sp_hoist[0] is not None:
                        sp = sp_hoist[0]
                    else:
                        sp = emit_sim(jt)
                    if hoist_sim and jt == 0 and pend["av"] is not None:
                        sp_hoist[0] = emit_sim(1)
                    eq = etp.tile([128, n], f16, tag="eq", bufs=8, name="eq")
                    nc.scalar.activation(eq[:], sp[:], Exp)
                    et = etp.tile([128, n], f16, tag="et", name="et")
                    use_pool = (not is_last) and jt in pool_jts
                    mul_eng = nc.gpsimd if use_pool else nc.vector
                    mul_eng.tensor_mul(out=et[:], in0=eq[:], in1=eb_tiles[(h, jt)])
                    et_row.append(et)
                    if is_last:
                        for ihh in range(2):
                            nc.tensor.matmul(
                                ot_last[:, ihh * 512:(ihh + 1) * 512],
                                VA[bi][jt][:, h * (DH + 1):(h + 1) * (DH + 1)],
                                et[:, ihh * 512:(ihh + 1) * 512],
                                start=(jt == 0), stop=(jt == NT - 1),
                            )
                    if jt == av_flush_slot:
                        step_av(99)
                    if slot in sched:
                        emit_chunk(sched.pop(slot))
                    slot += 1
                if is_last:
                    emit_norm(h, bi, ot_last)
                else:
                    emit_av(h, bi, et_row)
        step_av(99)
        # drain: anything left, then F for batch 1
        for s in sorted(sched):
            emit_chunk(sched[s])
        for t2 in range(T2):
            emit_f_chunk(1, t2, ring=("mm" if (f_drain_mm and t2 % 2) else None))

        for p in (ps, fop, lrp, etp, ebp, xtp, pers):
            p.release()

    nc.compile()
    return nc


def prep_inputs(x, Wq, Wk, Wv, rel_bias, Wo, bo, n=N, bpc=BPC, ncores=NCORES):
    """Host-side sharding/layout prep. Returns in_maps (one dict per core)."""
    f16 = np.float16
    x = np.asarray(x, dtype=np.float32)
    xT = np.ascontiguousarray(x.transpose(0, 2, 1)).astype(f16)   # [B, D, n]
    WqT = np.ascontiguousarray(Wq.T * np.float32(SCALE)).astype(f16)
    WkT = np.ascontiguousarray(Wk.T).astype(f16)
    WvT = np.ascontiguousarray(Wv.T).astype(f16)
    WoT = np.ascontiguousarray(Wo.T).astype(f16)
    expBT = np.ascontiguousarray(
        np.exp(np.asarray(rel_bias, dtype=np.float32).transpose(0, 2, 1))
    ).astype(f16)                                                  # [H, n(j), n(i)]
    in_maps = []
    for c in range(ncores):
        in_maps.append({
            "xT": np.ascontiguousarray(xT[c * bpc:(c + 1) * bpc]),
            "WqT": WqT, "WkT": WkT, "WvT": WvT, "WoT": WoT,
            "expBT": expBT,
        })
    return in_maps


_CACHE = {}


def kernel(x, Wq, Wk, Wv, rel_bias, Wo, bo):
    from concourse.bass_utils import run_bass_kernel_spmd

    if "nc" not in _CACHE:
        _CACHE["nc"] = build_nc()
    nc = _CACHE["nc"]
    in_maps = prep_inputs(x, Wq, Wk, Wv, rel_bias, Wo, bo)
    res = run_bass_kernel_spmd(nc, in_maps, core_ids=list(range(NCORES)))
    out = np.concatenate([res.results[c]["out"] for c in range(NCORES)], axis=0)
    out = out + np.asarray(bo, dtype=np.float32)[None, None, :]
    return np.ascontiguousarray(out, dtype=np.float32)


# revision 37
# speedup vs baseline: 1.2204x; 1.0028x over previous
"""CrossAttention kernel for 8x TRN2 NeuronCores (Bass/Tile), v2.

Reference computation (per batch b of 16, heads h=8, n=1024, d_model=512, dh=64):
    q = x @ Wq.T, k = x @ Wk.T, v = x @ Wv.T          (per-head slices)
    sim = q k^T * scale + rel_bias[h]
    attn = softmax(sim, axis=-1)
    out = (attn @ v) re-assembled over heads, then @ Wo.T + bo

Sharding: data-parallel over batch, 2 batches per core x 8 cores.

v2 design notes (vs v1 baseline at ~264us modeled):
  - all matmul operands fp16 (full-rate on PE at any tile size, better
    mantissa than bf16, halves weight/x/eb DMA vs f32).
  - softmax runs on transposed sim (j on partitions); rel_bias applied as
    exp(sim)*exp(bias^T) with exp(bias^T) precomputed on host in fp16;
    the multiply runs in-place on DVE in 16-bit 2x mode.
  - attn@V uses V in natural layout as lhsT with an appended ones column:
    the same matmul emits the softmax denominator l as psum row 64.
  - normalization (HW constraint: custom DVE ops and partition_broadcast
    only operate from partition 0, and only on SBUF): DVE-copy the psum
    l-row to SBUF, DMA it to partition 0, reciprocal_approx_fast there,
    gpsimd partition_broadcast, one 1024-wide DVE multiply. Odd heads land
    in AO rows 64..127 via one SBUF->SBUF DMA.
  - output is fp16 (host converts to f32 and adds bias bo) - halves the
    serial output-DMA drain at the end of the program.
  - PSUM split into two rings: "mm" (sim tiles only, so ACT never starves
    behind injected work) and "ot" (attn@V accumulators + all projection /
    output-projection chunks).
  - coarse DMAs: one descriptor-chain per weight matrix / x batch, rel-bias
    loaded in jt-pairs - halves serial HWDGE occupancy.
  - emission is software-pipelined: projections for ip0 first, then the
    attention stream loop with V/QK(ip1..3)/F chunks injected between
    sim slots so the PE never starves while ACT grinds exp.
"""

import numpy as np

HEADS = 8
DH = 64
B = 16
N = 1024
D = 512  # d_model = inner
SCALE = DH ** -0.5
NCORES = 8
BPC = B // NCORES  # batches per core


def build_nc(n=N, bpc=BPC, sim_bufs=2, ot_bufs=2, eb_bufs=7, et_bufs=18,
             lb_bufs=3, fo_bufs=6, tmpo_bufs=2, pool_mul_mod=0):
    import concourse.mybir as mybir
    import concourse.tile as tile
    from concourse import bacc

    f32 = mybir.dt.float32
    f16 = mybir.dt.float16
    Exp = mybir.ActivationFunctionType.Exp
    Copy = mybir.ActivationFunctionType.Copy

    NT = n // 128            # n tiles of 128 (8)
    KP = D // 128            # d_model k-tiles (4)
    HP = HEADS // 2          # head pairs (4)
    T2 = NT // 2             # double-nt chunks (4)

    nc = bacc.Bacc(None, target_bir_lowering=False)

    xT_d = nc.dram_tensor("xT", [bpc, D, n], f16, kind="ExternalInput")
    wq_d = nc.dram_tensor("WqT", [D, D], f16, kind="ExternalInput")   # pre-scaled
    wk_d = nc.dram_tensor("WkT", [D, D], f16, kind="ExternalInput")
    wv_d = nc.dram_tensor("WvT", [D, D], f16, kind="ExternalInput")
    wo_d = nc.dram_tensor("WoT", [D, D], f16, kind="ExternalInput")
    eb_d = nc.dram_tensor("expBT", [HEADS, n, n], f16, kind="ExternalInput")
    out_d = nc.dram_tensor("out", [bpc, n, D], f16, kind="ExternalOutput")

    with tile.TileContext(nc) as tc:
        pers = tc.alloc_tile_pool(name="pers", bufs=1)
        # ---- persistent tiles
        QT = [[pers.tile([128, n], f16, tag=f"qt{bi}_{ip}", name=f"qt{bi}_{ip}")
               for ip in range(KP)] for bi in range(bpc)]
        KT = [[pers.tile([128, n], f16, tag=f"kt{bi}_{ip}", name=f"kt{bi}_{ip}")
               for ip in range(KP)] for bi in range(bpc)]
        VA = [[pers.tile([128, HEADS * (DH + 1)], f16, tag=f"va{bi}_{nt}",
                         name=f"va{bi}_{nt}") for nt in range(NT)]
              for bi in range(bpc)]
        AO = [[pers.tile([128, n], f16, tag=f"ao{bi}_{kp}", name=f"ao{bi}_{kp}")
               for kp in range(KP)] for bi in range(bpc)]
        # each weight matrix lives in one [128, KP, 512] tile (one DMA each)
        w4 = {}
        for wname, wd in (("q", wq_d), ("k", wk_d), ("v", wv_d), ("o", wo_d)):
            t = pers.tile([128, KP, D], f16, tag=f"w{wname}", name=f"w{wname}")
            w4[wname] = t
        xtp = tc.alloc_tile_pool(name="xt", bufs=1)
        xt = [xtp.tile([128, KP, n], f16, tag=f"x{bi}", name=f"x{bi}")
              for bi in range(bpc)]

        ebp = tc.alloc_tile_pool(name="eb", bufs=eb_bufs)
        etp = tc.alloc_tile_pool(name="et", bufs=et_bufs)
        lrp = tc.alloc_tile_pool(name="lr", bufs=lb_bufs)
        fop = tc.alloc_tile_pool(name="fop", bufs=1)
        ps = tc.alloc_tile_pool(name="ps", bufs=1, space="PSUM")

        # ---- prologue DMAs (one chained DMA per tensor)
        def dma_w(wname, wd):
            dst = w4[wname][:]
            src = wd.rearrange("(kp p) c -> p kp c", p=128)
            nc.sync.dma_start(out=dst, in_=src)

        def dma_x(bi, split=False):
            if split:
                for kp in range(KP):
                    nc.sync.dma_start(
                        out=xt[bi][:, kp, :],
                        in_=xT_d[bi, kp * 128:(kp + 1) * 128, :])
            else:
                nc.sync.dma_start(
                    out=xt[bi][:], in_=xT_d[bi].rearrange("(kp p) j -> p kp j", p=128))

        pool_jts = set(range(3, 3 + pool_mul_mod))
        av_jt_order = [j for j in range(NT) if j not in pool_jts] + sorted(pool_jts)
        eb_tiles = {}

        def emit_eb_dma(h, jp):
            """Load jt pair (2*jp, 2*jp+1) of head h as one [128, 2, n] tile."""
            t = ebp.tile([128, 2, n], f16, tag="eb", name="eb")
            nc.sync.dma_start(
                out=t[:],
                in_=eb_d[h, 2 * jp * 128:(2 * jp + 2) * 128, :].rearrange(
                    "(two p) i -> p two i", two=2))
            eb_tiles[(h, 2 * jp)] = t[:, 0, :]
            eb_tiles[(h, 2 * jp + 1)] = t[:, 1, :]

        dma_w("q", wq_d)
        dma_x(0, split=bool(x0_split))
        dma_w("k", wk_d)
        dma_x(1)
        dma_w("v", wv_d)
        dma_w("o", wo_d)
        for jp in range(NT // 2):
            emit_eb_dma(0, jp)

        # ---- PE warm-up: keep the tensor engine continuously busy through
        # the prologue DMAs so its p-state clock is fully ramped (213ns/row
        # instead of 427+) when the first real projection matmuls arrive.
        if warmup_mms:
            scr = pers.tile([128, 512], f16, tag="scr", name="scr")
            nc.gpsimd.memset(scr[:], 0.0)
            wps = ps.tile([128, 512], f32, tag="mm", bufs=sim_bufs, name="wps")
            for _ in range(warmup_mms):
                nc.tensor.matmul(wps[:], scr[:, 0:128], scr[:],
                                 start=True, stop=True)

        # ---- emitters -----------------------------------------------------
        inj_ring = [0]

        def inj_tag():
            return "ot"

        def emit_qk_chunk(wname, DST, bi, ip, ring=None, halves=(0, 1)):
            W_s = w4[wname]
            wide = len(halves) == 2 and not split_inj
            pt = ps.tile([128, 1024 if wide else 512], f32,
                         tag=ring or inj_tag(), bufs=ot_bufs, name="pt")
            for i, nh in enumerate(halves):
                base = i * 512 if wide else 0
                for kp in range(KP):
                    nc.tensor.matmul(
                        pt[:, base:base + 512],
                        W_s[:, kp, ip * 128:(ip + 1) * 128],
                        xt[bi][:, kp, nh * 512:(nh + 1) * 512],
                        start=(kp == 0), stop=(kp == KP - 1),
                    )
                if not wide:
                    nc.vector.tensor_copy(
                        out=DST[bi][ip][:, nh * 512:(nh + 1) * 512], in_=pt[:])
            if wide:
                nc.vector.tensor_copy(out=DST[bi][ip][:], in_=pt[:])

        def emit_v_chunk(bi, t2):
            pt = ps.tile([128, 1024], f32, tag=inj_tag(), bufs=ot_bufs, name="pt")
            for b in range(2):
                nt = 2 * t2 + b
                for kp in range(KP):
                    nc.tensor.matmul(
                        pt[:, b * 512:(b + 1) * 512],
                        xt[bi][:, kp, nt * 128:(nt + 1) * 128],
                        w4["v"][:, kp, :],
                        start=(kp == 0), stop=(kp == KP - 1),
                    )
            for b in range(2):
                nt = 2 * t2 + b
                va = VA[bi][nt]
                nc.gpsimd.memset(va[:], 1.0)
                dst3 = va[:].rearrange("p (h c) -> p h c", c=DH + 1)[:, :, 0:DH]
                src3 = pt[:, b * 512:(b + 1) * 512].rearrange("p (h c) -> p h c", c=DH)
                nc.vector.tensor_copy(out=dst3, in_=src3)

        def emit_f_chunk(bi, t2, ring=None):
            pt = ps.tile([128, 1024], f32, tag=ring or inj_tag(), bufs=ot_bufs,
                         name="pt")
            for b in range(2):
                nt = 2 * t2 + b
                for kp in range(KP):
                    nc.tensor.matmul(
                        pt[:, b * 512:(b + 1) * 512],
                        AO[bi][kp][:, nt * 128:(nt + 1) * 128],
                        w4["o"][:, kp, :],
                        start=(kp == 0), stop=(kp == KP - 1),
                    )
            for b in range(2):
                nt = 2 * t2 + b
                fo = fop.tile([128, 512], f16, tag="fo", bufs=fo_bufs, name="fo")
                nc.scalar.activation(fo[:], pt[:, b * 512:(b + 1) * 512], Copy)
                nc.sync.dma_start(out=out_d[bi, nt * 128:(nt + 1) * 128, :], in_=fo[:])

        pend = {"av": None}

        def start_av(h, bi, et_row):
            ot = ps.tile([DH + 1, 1024], f32, tag="ot", bufs=ot_bufs, name="ot")
            mms = [(ihh, jt) for ihh in range(2) for jt in av_jt_order]
            pend["av"] = {"h": h, "bi": bi, "et": et_row, "ot": ot, "mms": mms}

        def step_av(nmm):
            st = pend["av"]
            if st is None:
                return
            h, bi, et_row, ot = st["h"], st["bi"], st["et"], st["ot"]
            while nmm > 0 and st["mms"]:
                ihh, jt = st["mms"].pop(0)
                nc.tensor.matmul(
                    ot[:, ihh * 512:(ihh + 1) * 512],
                    VA[bi][jt][:, h * (DH + 1):(h + 1) * (DH + 1)],
                    et_row[jt][:, ihh * 512:(ihh + 1) * 512],
                    start=(jt == av_jt_order[0]), stop=(jt == av_jt_order[-1]),
                )
                nmm -= 1
            if not st["mms"]:
                emit_norm(h, bi, ot)
                pend["av"] = None

        def emit_norm(h, bi, ot):
            # reciprocal straight off the psum l-row into row 64 of the
            # broadcast tile, gpsimd broadcast from partition 64.
            lb = lrp.tile([DH + 1, n], f32, tag="lb", name="lb")
            nc.vector.reciprocal_approx_fast(out=lb[DH:DH + 1, :], in_=ot[DH:DH + 1, :])
            nc.gpsimd.partition_broadcast(lb[0:DH, :], lb[DH:DH + 1, :], channels=DH)
            if h % 2 == 0:
                nc.vector.tensor_mul(
                    out=AO[bi][h // 2][0:DH, :], in0=ot[0:DH, :], in1=lb[0:DH, :])
            else:
                tmpo = lrp.tile([DH, n], f16, tag="tmpo", bufs=tmpo_bufs, name="tmpo")
                nc.vector.tensor_mul(out=tmpo[:], in0=ot[0:DH, :], in1=lb[0:DH, :])
                nc.sync.dma_start(out=AO[bi][h // 2][DH:128, :], in_=tmpo[:])

        # ---- prologue compute: QK projections for ip=0 (heads 0,1).
        # The sim ring is empty this early, so borrow it: four chunks then
        # rotate through four psum slots instead of two.
        for bi in range(bpc):
            emit_qk_chunk("q", QT, bi, 0, ring="mm" if proq_mm else None)
            emit_qk_chunk("k", KT, bi, 0, ring="mm" if proq_mm else None)

        # ---- deadline-scheduled inject: chunk -> emission slot
        sched = {}
        slots_v0 = [0, 2, 4, 6]
        slots_v1 = [8, 10, 12, 14]
        slots_ip = {1: [18, 21, 24, 27], 2: [36, 44, 52, 60], 3: [68, 76, 84, 92]}
        if split_inj:
            for t2 in range(T2):
                sched[slots_v0[t2]] = [("vh", 0, t2, 0), ("vh", 0, t2, 1)]
                sched[slots_v1[t2]] = [("vh", 1, t2, 0), ("vh", 1, t2, 1)]
            for ip in range(1, KP):
                for i, (kind, bi) in enumerate((("q", 0), ("k", 0), ("q", 1), ("k", 1))):
                    sched[slots_ip[ip][i]] = [(kind + "h", bi, ip, 0),
                                              (kind + "h", bi, ip, 1)]
        else:
            for t2 in range(T2):
                sched[slots_v0[t2]] = [("v", 0, t2)]
                sched[slots_v1[t2]] = [("v", 1, t2)]
            for ip in range(1, KP):
                for i, (kind, bi) in enumerate((("q", 0), ("k", 0), ("q", 1), ("k", 1))):
                    sched[slots_ip[ip][i]] = [(kind, bi, ip)]
        for t2 in range(T2):
            sched[120 + 2 * t2] = [("f", 0, t2)]

        def emit_chunk(c):
            kind = c[0]
            if kind == "v":
                emit_v_chunk(c[1], c[2])
            elif kind == "vh":
                emit_v_chunk(c[1], c[2], blocks=(c[3],))
            elif kind == "q":
                emit_qk_chunk("q", QT, c[1], c[2])
            elif kind == "k":
                emit_qk_chunk("k", KT, c[1], c[2])
            elif kind == "qh":
                emit_qk_chunk("q", QT, c[1], c[2], halves=(c[3],))
            elif kind == "kh":
                emit_qk_chunk("k", KT, c[1], c[2], halves=(c[3],))
            elif kind == "f":
                emit_f_chunk(c[1], c[2])

        # ---- main attention loop; last head pair runs h-odd first so the
        # final stream is an even head (its norm-mul writes AO directly,
        # shortening the tail by one SBUF->SBUF DMA hop).
        stream_order = {}
        for hp in range(HP):
            h0, h1 = 2 * hp, 2 * hp + 1
            if hp == HP - 1:
                stream_order[hp] = [(h1, 0), (h1, 1), (h0, 0), (h0, 1)]
            else:
                stream_order[hp] = [(h0, 0), (h0, 1), (h1, 0), (h1, 1)]

        slot = 0
        for hp in range(HP):
            streams = stream_order[hp]
            first_h = streams[0][0]
            other_h = streams[2][0]
            for sidx, (h, bi) in enumerate(streams):
                is_last = (hp == HP - 1 and sidx == 3)
                ot_last = None
                if is_last:
                    ot_last = ps.tile([DH + 1, 1024], f32, tag="ot",
                                      bufs=ot_bufs, name="ot_last")
                et_row = []
                sp_hoist = [None]
                for jt in range(NT):
                    # eb prefetch (jt pairs): sidx1 fetches this pair's other
                    # head, sidx2 fetches the next head-pair's first head.
                    if jt % 2 == 0:
                        if sidx == 1:
                            emit_eb_dma(other_h, jt // 2)
                        elif sidx == 2 and hp + 1 < HP:
                            emit_eb_dma(stream_order[hp + 1][0][0], jt // 2)
                    po = (h % 2) * 64

                    def emit_sim(j):
                        spj = ps.tile([128, 1024], f32, tag="mm", bufs=sim_bufs,
                                      name="spj")
                        for ihh in range(2):
                            nc.tensor.matmul(
                                spj[:, ihh * 512:(ihh + 1) * 512],
                                KT[bi][hp][po:po + 64, j * 128:(j + 1) * 128],
                                QT[bi][hp][po:po + 64, ihh * 512:(ihh + 1) * 512],
                                start=True, stop=True,
                            )
                        return spj

                    if jt == 1 and sp_hoist[0] is not None:
                        sp = sp_hoist[0]
                    else:
                        sp = emit_sim(jt)
                    if hoist_sim and jt == 0 and pend["av"] is not None:
                        sp_hoist[0] = emit_sim(1)
                    eq = etp.tile([128, n], f16, tag="eq", bufs=8, name="eq")
                    nc.scalar.activation(eq[:], sp[:], Exp)
                    et = etp.tile([128, n], f16, tag="et", name="et")
                    use_pool = (not is_last) and jt in pool_jts
                    mul_eng = nc.gpsimd if use_pool else nc.vector
                    mul_eng.tensor_mul(out=et[:], in0=eq[:], in1=eb_tiles[(h, jt)])
                    et_row.append(et)
                    if is_last:
                        for ihh in range(2):
                            nc.tensor.matmul(
                                ot_last[:, ihh * 512:(ihh + 1) * 512],
                                VA[bi][jt][:, h * (DH + 1):(h + 1) * (DH + 1)],
                                et[:, ihh * 512:(ihh + 1) * 512],
                                start=(jt == 0), stop=(jt == NT - 1),
                            )
                    if jt == av_flush_slot:
                        step_av(99)
                    if slot in sched:
                        for c in sched.pop(slot):
                            emit_chunk(c)
                    slot += 1
                if is_last:
                    emit_norm(h, bi, ot_last)
                else:
                    emit_av(h, bi, et_row)
        step_av(99)
        # drain: anything left, then F for batch 1
        for s in sorted(sched):
            for c in sched[s]:
                emit_chunk(c)
        for t2 in range(T2):
            emit_f_chunk(1, t2, ring=("mm" if (f_drain_mm and t2 % 2) else None))

        for p in (ps, fop, lrp, etp, ebp, xtp, pers):
            p.release()

    nc.compile()
    return nc


def prep_inputs(x, Wq, Wk, Wv, rel_bias, Wo, bo, n=N, bpc=BPC, ncores=NCORES):
    """Host-side sharding/layout prep. Returns in_maps (one dict per core)."""
    f16 = np.float16
    x = np.asarray(x, dtype=np.float32)
    xT = np.ascontiguousarray(x.transpose(0, 2, 1)).astype(f16)   # [B, D, n]
    WqT = np.ascontiguousarray(Wq.T * np.float32(SCALE)).astype(f16)
    WkT = np.ascontiguousarray(Wk.T).astype(f16)
    WvT = np.ascontiguousarray(Wv.T).astype(f16)
    WoT = np.ascontiguousarray(Wo.T).astype(f16)
    expBT = np.ascontiguousarray(
        np.exp(np.asarray(rel_bias, dtype=np.float32).transpose(0, 2, 1))
    ).astype(f16)                                                  # [H, n(j), n(i)]
    in_maps = []
    for c in range(ncores):
        in_maps.append({
            "xT": np.ascontiguousarray(xT[c * bpc:(c + 1) * bpc]),
            "WqT": WqT, "WkT": WkT, "WvT": WvT, "WoT": WoT,
            "expBT": expBT,
        })
    return in_maps


_CACHE = {}


def kernel(x, Wq, Wk, Wv, rel_bias, Wo, bo):
    from concourse.bass_utils import run_bass_kernel_spmd

    if "nc" not in _CACHE:
        _CACHE["nc"] = build_nc()
    nc = _CACHE["nc"]
    in_maps = prep_inputs(x, Wq, Wk, Wv, rel_bias, Wo, bo)
    res = run_bass_kernel_spmd(nc, in_maps, core_ids=list(range(NCORES)))
    out = np.concatenate([res.results[c]["out"] for c in range(NCORES)], axis=0)
    out = out + np.asarray(bo, dtype=np.float32)[None, None, :]
    return np.ascontiguousarray(out, dtype=np.float32)


# revision 38
# speedup vs baseline: 1.2219x; 1.0013x over previous
"""CrossAttention kernel for 8x TRN2 NeuronCores (Bass/Tile), v2.

Reference computation (per batch b of 16, heads h=8, n=1024, d_model=512, dh=64):
    q = x @ Wq.T, k = x @ Wk.T, v = x @ Wv.T          (per-head slices)
    sim = q k^T * scale + rel_bias[h]
    attn = softmax(sim, axis=-1)
    out = (attn @ v) re-assembled over heads, then @ Wo.T + bo

Sharding: data-parallel over batch, 2 batches per core x 8 cores.

v2 design notes (vs v1 baseline at ~264us modeled):
  - all matmul operands fp16 (full-rate on PE at any tile size, better
    mantissa than bf16, halves weight/x/eb DMA vs f32).
  - softmax runs on transposed sim (j on partitions); rel_bias applied as
    exp(sim)*exp(bias^T) with exp(bias^T) precomputed on host in fp16;
    the multiply runs in-place on DVE in 16-bit 2x mode.
  - attn@V uses V in natural layout as lhsT with an appended ones column:
    the same matmul emits the softmax denominator l as psum row 64.
  - normalization (HW constraint: custom DVE ops and partition_broadcast
    only operate from partition 0, and only on SBUF): DVE-copy the psum
    l-row to SBUF, DMA it to partition 0, reciprocal_approx_fast there,
    gpsimd partition_broadcast, one 1024-wide DVE multiply. Odd heads land
    in AO rows 64..127 via one SBUF->SBUF DMA.
  - output is fp16 (host converts to f32 and adds bias bo) - halves the
    serial output-DMA drain at the end of the program.
  - PSUM split into two rings: "mm" (sim tiles only, so ACT never starves
    behind injected work) and "ot" (attn@V accumulators + all projection /
    output-projection chunks).
  - coarse DMAs: one descriptor-chain per weight matrix / x batch, rel-bias
    loaded in jt-pairs - halves serial HWDGE occupancy.
  - emission is software-pipelined: projections for ip0 first, then the
    attention stream loop with V/QK(ip1..3)/F chunks injected between
    sim slots so the PE never starves while ACT grinds exp.
"""

import numpy as np

HEADS = 8
DH = 64
B = 16
N = 1024
D = 512  # d_model = inner
SCALE = DH ** -0.5
NCORES = 8
BPC = B // NCORES  # batches per core


def build_nc(n=N, bpc=BPC, sim_bufs=2, ot_bufs=2, eb_bufs=7, et_bufs=18,
             lb_bufs=3, fo_bufs=6, tmpo_bufs=2, pool_mul_mod=0):
    import concourse.mybir as mybir
    import concourse.tile as tile
    from concourse import bacc

    f32 = mybir.dt.float32
    f16 = mybir.dt.float16
    Exp = mybir.ActivationFunctionType.Exp
    Copy = mybir.ActivationFunctionType.Copy

    NT = n // 128            # n tiles of 128 (8)
    KP = D // 128            # d_model k-tiles (4)
    HP = HEADS // 2          # head pairs (4)
    T2 = NT // 2             # double-nt chunks (4)

    nc = bacc.Bacc(None, target_bir_lowering=False)

    xT_d = nc.dram_tensor("xT", [bpc, D, n], f16, kind="ExternalInput")
    wq_d = nc.dram_tensor("WqT", [D, D], f16, kind="ExternalInput")   # pre-scaled
    wk_d = nc.dram_tensor("WkT", [D, D], f16, kind="ExternalInput")
    wv_d = nc.dram_tensor("WvT", [D, D], f16, kind="ExternalInput")
    wo_d = nc.dram_tensor("WoT", [D, D], f16, kind="ExternalInput")
    eb_d = nc.dram_tensor("expBT", [HEADS, n, n], f16, kind="ExternalInput")
    out_d = nc.dram_tensor("out", [bpc, n, D], f16, kind="ExternalOutput")

    with tile.TileContext(nc) as tc:
        pers = tc.alloc_tile_pool(name="pers", bufs=1)
        # ---- persistent tiles
        QT = [[pers.tile([128, n], f16, tag=f"qt{bi}_{ip}", name=f"qt{bi}_{ip}")
               for ip in range(KP)] for bi in range(bpc)]
        KT = [[pers.tile([128, n], f16, tag=f"kt{bi}_{ip}", name=f"kt{bi}_{ip}")
               for ip in range(KP)] for bi in range(bpc)]
        VA = [[pers.tile([128, HEADS * (DH + 1)], f16, tag=f"va{bi}_{nt}",
                         name=f"va{bi}_{nt}") for nt in range(NT)]
              for bi in range(bpc)]
        AO = [[pers.tile([128, n], f16, tag=f"ao{bi}_{kp}", name=f"ao{bi}_{kp}")
               for kp in range(KP)] for bi in range(bpc)]
        # each weight matrix lives in one [128, KP, 512] tile (one DMA each)
        w4 = {}
        for wname, wd in (("q", wq_d), ("k", wk_d), ("v", wv_d), ("o", wo_d)):
            t = pers.tile([128, KP, D], f16, tag=f"w{wname}", name=f"w{wname}")
            w4[wname] = t
        xtp = tc.alloc_tile_pool(name="xt", bufs=1)
        xt = [xtp.tile([128, KP, n], f16, tag=f"x{bi}", name=f"x{bi}")
              for bi in range(bpc)]

        ebp = tc.alloc_tile_pool(name="eb", bufs=eb_bufs)
        etp = tc.alloc_tile_pool(name="et", bufs=et_bufs)
        lrp = tc.alloc_tile_pool(name="lr", bufs=lb_bufs)
        fop = tc.alloc_tile_pool(name="fop", bufs=1)
        ps = tc.alloc_tile_pool(name="ps", bufs=1, space="PSUM")

        # ---- prologue DMAs (one chained DMA per tensor)
        def dma_w(wname, wd):
            dst = w4[wname][:]
            src = wd.rearrange("(kp p) c -> p kp c", p=128)
            nc.sync.dma_start(out=dst, in_=src)

        def dma_x(bi, split=False):
            if split:
                for kp in range(KP):
                    nc.sync.dma_start(
                        out=xt[bi][:, kp, :],
                        in_=xT_d[bi, kp * 128:(kp + 1) * 128, :])
            else:
                nc.sync.dma_start(
                    out=xt[bi][:], in_=xT_d[bi].rearrange("(kp p) j -> p kp j", p=128))

        pool_jts = set(range(3, 3 + pool_mul_mod))
        av_jt_order = [j for j in range(NT) if j not in pool_jts] + sorted(pool_jts)
        eb_tiles = {}

        def emit_eb_dma(h, jp):
            """Load jt pair (2*jp, 2*jp+1) of head h as one [128, 2, n] tile."""
            t = ebp.tile([128, 2, n], f16, tag="eb", name="eb")
            nc.sync.dma_start(
                out=t[:],
                in_=eb_d[h, 2 * jp * 128:(2 * jp + 2) * 128, :].rearrange(
                    "(two p) i -> p two i", two=2))
            eb_tiles[(h, 2 * jp)] = t[:, 0, :]
            eb_tiles[(h, 2 * jp + 1)] = t[:, 1, :]

        dma_w("q", wq_d)
        dma_x(0, split=bool(x0_split))
        dma_w("k", wk_d)
        dma_x(1)
        dma_w("v", wv_d)
        dma_w("o", wo_d)
        for jp in range(NT // 2):
            emit_eb_dma(0, jp)

        # ---- PE warm-up: keep the tensor engine continuously busy through
        # the prologue DMAs so its p-state clock is fully ramped (213ns/row
        # instead of 427+) when the first real projection matmuls arrive.
        if warmup_mms:
            scr = pers.tile([128, 512], f16, tag="scr", name="scr")
            nc.gpsimd.memset(scr[:], 0.0)
            wps = ps.tile([128, 512], f32, tag="mm", bufs=sim_bufs, name="wps")
            for _ in range(warmup_mms):
                nc.tensor.matmul(wps[:], scr[:, 0:128], scr[:],
                                 start=True, stop=True)

        # ---- emitters -----------------------------------------------------
        inj_ring = [0]

        def inj_tag():
            return "ot"

        def emit_qk_chunk(wname, DST, bi, ip, ring=None, halves=(0, 1), first_split=False):
            W_s = w4[wname]
            wide = len(halves) == 2 and not split_inj
            pt = ps.tile([128, 1024 if wide else 512], f32,
                         tag=ring or inj_tag(), bufs=ot_bufs, name="pt")
            for i, nh in enumerate(halves):
                base = i * 512 if wide else 0
                for kp in range(KP):
                    nc.tensor.matmul(
                        pt[:, base:base + 512],
                        W_s[:, kp, ip * 128:(ip + 1) * 128],
                        xt[bi][:, kp, nh * 512:(nh + 1) * 512],
                        start=(kp == 0), stop=(kp == KP - 1),
                    )
                if not wide:
                    nc.vector.tensor_copy(
                        out=DST[bi][ip][:, nh * 512:(nh + 1) * 512], in_=pt[:])
            if wide:
                if first_split:
                    # sim jt0 needs only K cols 0:128 / Q cols 0:512 - copy
                    # those first so the first sim fires sooner.
                    c0 = 128 if wname == "k" else 512
                    nc.vector.tensor_copy(out=DST[bi][ip][:, 0:c0], in_=pt[:, 0:c0])
                    nc.vector.tensor_copy(out=DST[bi][ip][:, c0:], in_=pt[:, c0:])
                else:
                    nc.vector.tensor_copy(out=DST[bi][ip][:], in_=pt[:])

        def emit_v_chunk(bi, t2):
            pt = ps.tile([128, 1024], f32, tag=inj_tag(), bufs=ot_bufs, name="pt")
            for b in range(2):
                nt = 2 * t2 + b
                for kp in range(KP):
                    nc.tensor.matmul(
                        pt[:, b * 512:(b + 1) * 512],
                        xt[bi][:, kp, nt * 128:(nt + 1) * 128],
                        w4["v"][:, kp, :],
                        start=(kp == 0), stop=(kp == KP - 1),
                    )
            for b in range(2):
                nt = 2 * t2 + b
                va = VA[bi][nt]
                nc.gpsimd.memset(va[:], 1.0)
                dst3 = va[:].rearrange("p (h c) -> p h c", c=DH + 1)[:, :, 0:DH]
                src3 = pt[:, b * 512:(b + 1) * 512].rearrange("p (h c) -> p h c", c=DH)
                nc.vector.tensor_copy(out=dst3, in_=src3)

        def emit_f_chunk(bi, t2, ring=None):
            pt = ps.tile([128, 1024], f32, tag=ring or inj_tag(), bufs=ot_bufs,
                         name="pt")
            for b in range(2):
                nt = 2 * t2 + b
                for kp in range(KP):
                    nc.tensor.matmul(
                        pt[:, b * 512:(b + 1) * 512],
                        AO[bi][kp][:, nt * 128:(nt + 1) * 128],
                        w4["o"][:, kp, :],
                        start=(kp == 0), stop=(kp == KP - 1),
                    )
            for b in range(2):
                nt = 2 * t2 + b
                fo = fop.tile([128, 512], f16, tag="fo", bufs=fo_bufs, name="fo")
                nc.scalar.activation(fo[:], pt[:, b * 512:(b + 1) * 512], Copy)
                nc.sync.dma_start(out=out_d[bi, nt * 128:(nt + 1) * 128, :], in_=fo[:])

        pend = {"av": None}

        def start_av(h, bi, et_row):
            ot = ps.tile([DH + 1, 1024], f32, tag="ot", bufs=ot_bufs, name="ot")
            mms = [(ihh, jt) for ihh in range(2) for jt in av_jt_order]
            pend["av"] = {"h": h, "bi": bi, "et": et_row, "ot": ot, "mms": mms}

        def step_av(nmm):
            st = pend["av"]
            if st is None:
                return
            h, bi, et_row, ot = st["h"], st["bi"], st["et"], st["ot"]
            while nmm > 0 and st["mms"]:
                ihh, jt = st["mms"].pop(0)
                nc.tensor.matmul(
                    ot[:, ihh * 512:(ihh + 1) * 512],
                    VA[bi][jt][:, h * (DH + 1):(h + 1) * (DH + 1)],
                    et_row[jt][:, ihh * 512:(ihh + 1) * 512],
                    start=(jt == av_jt_order[0]), stop=(jt == av_jt_order[-1]),
                )
                nmm -= 1
            if not st["mms"]:
                emit_norm(h, bi, ot)
                pend["av"] = None

        def emit_norm(h, bi, ot):
            # reciprocal straight off the psum l-row into row 64 of the
            # broadcast tile, gpsimd broadcast from partition 64.
            lb = lrp.tile([DH + 1, n], f32, tag="lb", name="lb")
            nc.vector.reciprocal_approx_fast(out=lb[DH:DH + 1, :], in_=ot[DH:DH + 1, :])
            nc.gpsimd.partition_broadcast(lb[0:DH, :], lb[DH:DH + 1, :], channels=DH)
            if h % 2 == 0:
                nc.vector.tensor_mul(
                    out=AO[bi][h // 2][0:DH, :], in0=ot[0:DH, :], in1=lb[0:DH, :])
            else:
                tmpo = lrp.tile([DH, n], f16, tag="tmpo", bufs=tmpo_bufs, name="tmpo")
                nc.vector.tensor_mul(out=tmpo[:], in0=ot[0:DH, :], in1=lb[0:DH, :])
                nc.sync.dma_start(out=AO[bi][h // 2][DH:128, :], in_=tmpo[:])

        # ---- prologue compute: QK projections for ip=0 (heads 0,1).
        # The sim ring is empty this early, so borrow it: four chunks then
        # rotate through four psum slots instead of two.
        for bi in range(bpc):
            emit_qk_chunk("q", QT, bi, 0, ring="mm" if proq_mm else None,
                          first_split=(bi == 0 and bool(head_split)))
            emit_qk_chunk("k", KT, bi, 0, ring="mm" if proq_mm else None,
                          first_split=(bi == 0 and bool(head_split)))

        # ---- deadline-scheduled inject: chunk -> emission slot
        sched = {}
        slots_v0 = [0, 2, 4, 6]
        slots_v1 = [8, 10, 12, 14]
        slots_ip = {1: [18, 21, 24, 27], 2: [36, 44, 52, 60], 3: [68, 76, 84, 92]}
        if split_inj:
            for t2 in range(T2):
                sched[slots_v0[t2]] = [("vh", 0, t2, 0), ("vh", 0, t2, 1)]
                sched[slots_v1[t2]] = [("vh", 1, t2, 0), ("vh", 1, t2, 1)]
            for ip in range(1, KP):
                for i, (kind, bi) in enumerate((("q", 0), ("k", 0), ("q", 1), ("k", 1))):
                    sched[slots_ip[ip][i]] = [(kind + "h", bi, ip, 0),
                                              (kind + "h", bi, ip, 1)]
        else:
            for t2 in range(T2):
                sched[slots_v0[t2]] = [("v", 0, t2)]
                sched[slots_v1[t2]] = [("v", 1, t2)]
            for ip in range(1, KP):
                for i, (kind, bi) in enumerate((("q", 0), ("k", 0), ("q", 1), ("k", 1))):
                    sched[slots_ip[ip][i]] = [(kind, bi, ip)]
        for t2 in range(T2):
            sched[120 + 2 * t2] = [("f", 0, t2)]

        def emit_chunk(c):
            kind = c[0]
            if kind == "v":
                emit_v_chunk(c[1], c[2])
            elif kind == "vh":
                emit_v_chunk(c[1], c[2], blocks=(c[3],))
            elif kind == "q":
                emit_qk_chunk("q", QT, c[1], c[2])
            elif kind == "k":
                emit_qk_chunk("k", KT, c[1], c[2])
            elif kind == "qh":
                emit_qk_chunk("q", QT, c[1], c[2], halves=(c[3],))
            elif kind == "kh":
                emit_qk_chunk("k", KT, c[1], c[2], halves=(c[3],))
            elif kind == "f":
                emit_f_chunk(c[1], c[2])

        # ---- main attention loop; last head pair runs h-odd first so the
        # final stream is an even head (its norm-mul writes AO directly,
        # shortening the tail by one SBUF->SBUF DMA hop).
        stream_order = {}
        for hp in range(HP):
            h0, h1 = 2 * hp, 2 * hp + 1
            if hp == HP - 1:
                stream_order[hp] = [(h1, 0), (h1, 1), (h0, 0), (h0, 1)]
            else:
                stream_order[hp] = [(h0, 0), (h0, 1), (h1, 0), (h1, 1)]

        slot = 0
        for hp in range(HP):
            streams = stream_order[hp]
            first_h = streams[0][0]
            other_h = streams[2][0]
            for sidx, (h, bi) in enumerate(streams):
                is_last = (hp == HP - 1 and sidx == 3)
                ot_last = None
                if is_last:
                    ot_last = ps.tile([DH + 1, 1024], f32, tag="ot",
                                      bufs=ot_bufs, name="ot_last")
                et_row = []
                sp_hoist = [None]
                for jt in range(NT):
                    # eb prefetch (jt pairs): sidx1 fetches this pair's other
                    # head, sidx2 fetches the next head-pair's first head.
                    if jt % 2 == 0:
                        if sidx == 1:
                            emit_eb_dma(other_h, jt // 2)
                        elif sidx == 2 and hp + 1 < HP:
                            emit_eb_dma(stream_order[hp + 1][0][0], jt // 2)
                    po = (h % 2) * 64

                    def emit_sim(j):
                        spj = ps.tile([128, 1024], f32, tag="mm", bufs=sim_bufs,
                                      name="spj")
                        for ihh in range(2):
                            nc.tensor.matmul(
                                spj[:, ihh * 512:(ihh + 1) * 512],
                                KT[bi][hp][po:po + 64, j * 128:(j + 1) * 128],
                                QT[bi][hp][po:po + 64, ihh * 512:(ihh + 1) * 512],
                                start=True, stop=True,
                            )
                        return spj

                    if jt == 1 and sp_hoist[0] is not None:
                        sp = sp_hoist[0]
                    else:
                        sp = emit_sim(jt)
                    if hoist_sim and jt == 0 and pend["av"] is not None:
                        sp_hoist[0] = emit_sim(1)
                    eq = etp.tile([128, n], f16, tag="eq", bufs=8, name="eq")
                    nc.scalar.activation(eq[:], sp[:], Exp)
                    et = etp.tile([128, n], f16, tag="et", name="et")
                    use_pool = (not is_last) and jt in pool_jts
                    mul_eng = nc.gpsimd if use_pool else nc.vector
                    mul_eng.tensor_mul(out=et[:], in0=eq[:], in1=eb_tiles[(h, jt)])
                    et_row.append(et)
                    if is_last:
                        for ihh in range(2):
                            nc.tensor.matmul(
                                ot_last[:, ihh * 512:(ihh + 1) * 512],
                                VA[bi][jt][:, h * (DH + 1):(h + 1) * (DH + 1)],
                                et[:, ihh * 512:(ihh + 1) * 512],
                                start=(jt == 0), stop=(jt == NT - 1),
                            )
                    if jt == av_flush_slot:
                        step_av(99)
                    if slot in sched:
                        for c in sched.pop(slot):
                            emit_chunk(c)
                    slot += 1
                if is_last:
                    emit_norm(h, bi, ot_last)
                else:
                    emit_av(h, bi, et_row)
        step_av(99)
        # drain: anything left, then F for batch 1
        for s in sorted(sched):
            for c in sched[s]:
                emit_chunk(c)
        for t2 in range(T2):
            emit_f_chunk(1, t2, ring=("mm" if (f_drain_mm and t2 % 2) else None))

        for p in (ps, fop, lrp, etp, ebp, xtp, pers):
            p.release()

    nc.compile()
    return nc


def prep_inputs(x, Wq, Wk, Wv, rel_bias, Wo, bo, n=N, bpc=BPC, ncores=NCORES):
    """Host-side sharding/layout prep. Returns in_maps (one dict per core)."""
    f16 = np.float16
    x = np.asarray(x, dtype=np.float32)
    xT = np.ascontiguousarray(x.transpose(0, 2, 1)).astype(f16)   # [B, D, n]
    WqT = np.ascontiguousarray(Wq.T * np.float32(SCALE)).astype(f16)
    WkT = np.ascontiguousarray(Wk.T).astype(f16)
    WvT = np.ascontiguousarray(Wv.T).astype(f16)
    WoT = np.ascontiguousarray(Wo.T).astype(f16)
    expBT = np.ascontiguousarray(
        np.exp(np.asarray(rel_bias, dtype=np.float32).transpose(0, 2, 1))
    ).astype(f16)                                                  # [H, n(j), n(i)]
    in_maps = []
    for c in range(ncores):
        in_maps.append({
            "xT": np.ascontiguousarray(xT[c * bpc:(c + 1) * bpc]),
            "WqT": WqT, "WkT": WkT, "WvT": WvT, "WoT": WoT,
            "expBT": expBT,
        })
    return in_maps


_CACHE = {}


def kernel(x, Wq, Wk, Wv, rel_bias, Wo, bo):
    from concourse.bass_utils import run_bass_kernel_spmd

    if "nc" not in _CACHE:
        _CACHE["nc"] = build_nc()
    nc = _CACHE["nc"]
    in_maps = prep_inputs(x, Wq, Wk, Wv, rel_bias, Wo, bo)
    res = run_bass_kernel_spmd(nc, in_maps, core_ids=list(range(NCORES)))
    out = np.concatenate([res.results[c]["out"] for c in range(NCORES)], axis=0)
    out = out + np.asarray(bo, dtype=np.float32)[None, None, :]
    return np.ascontiguousarray(out, dtype=np.float32)


# revision 39
# speedup vs baseline: 1.2297x; 1.0064x over previous
"""CrossAttention kernel for 8x TRN2 NeuronCores (Bass/Tile), v2.

Reference computation (per batch b of 16, heads h=8, n=1024, d_model=512, dh=64):
    q = x @ Wq.T, k = x @ Wk.T, v = x @ Wv.T          (per-head slices)
    sim = q k^T * scale + rel_bias[h]
    attn = softmax(sim, axis=-1)
    out = (attn @ v) re-assembled over heads, then @ Wo.T + bo

Sharding: data-parallel over batch, 2 batches per core x 8 cores.

v2 design notes (vs v1 baseline at ~264us modeled):
  - all matmul operands fp16 (full-rate on PE at any tile size, better
    mantissa than bf16, halves weight/x/eb DMA vs f32).
  - softmax runs on transposed sim (j on partitions); rel_bias applied as
    exp(sim)*exp(bias^T) with exp(bias^T) precomputed on host in fp16;
    the multiply runs in-place on DVE in 16-bit 2x mode.
  - attn@V uses V in natural layout as lhsT with an appended ones column:
    the same matmul emits the softmax denominator l as psum row 64.
  - normalization (HW constraint: custom DVE ops and partition_broadcast
    only operate from partition 0, and only on SBUF): DVE-copy the psum
    l-row to SBUF, DMA it to partition 0, reciprocal_approx_fast there,
    gpsimd partition_broadcast, one 1024-wide DVE multiply. Odd heads land
    in AO rows 64..127 via one SBUF->SBUF DMA.
  - output is fp16 (host converts to f32 and adds bias bo) - halves the
    serial output-DMA drain at the end of the program.
  - PSUM split into two rings: "mm" (sim tiles only, so ACT never starves
    behind injected work) and "ot" (attn@V accumulators + all projection /
    output-projection chunks).
  - coarse DMAs: one descriptor-chain per weight matrix / x batch, rel-bias
    loaded in jt-pairs - halves serial HWDGE occupancy.
  - emission is software-pipelined: projections for ip0 first, then the
    attention stream loop with V/QK(ip1..3)/F chunks injected between
    sim slots so the PE never starves while ACT grinds exp.
"""

import numpy as np

HEADS = 8
DH = 64
B = 16
N = 1024
D = 512  # d_model = inner
SCALE = DH ** -0.5
NCORES = 8
BPC = B // NCORES  # batches per core


def build_nc(n=N, bpc=BPC, sim_bufs=2, ot_bufs=2, eb_bufs=7, et_bufs=18,
             lb_bufs=3, fo_bufs=6, tmpo_bufs=2, pool_mul_mod=0):
    import concourse.mybir as mybir
    import concourse.tile as tile
    from concourse import bacc

    f32 = mybir.dt.float32
    f16 = mybir.dt.float16
    Exp = mybir.ActivationFunctionType.Exp
    Copy = mybir.ActivationFunctionType.Copy

    NT = n // 128            # n tiles of 128 (8)
    KP = D // 128            # d_model k-tiles (4)
    HP = HEADS // 2          # head pairs (4)
    T2 = NT // 2             # double-nt chunks (4)

    nc = bacc.Bacc(None, target_bir_lowering=False)

    xT_d = nc.dram_tensor("xT", [bpc, D, n], f16, kind="ExternalInput")
    wq_d = nc.dram_tensor("WqT", [D, D], f16, kind="ExternalInput")   # pre-scaled
    wk_d = nc.dram_tensor("WkT", [D, D], f16, kind="ExternalInput")
    wv_d = nc.dram_tensor("WvT", [D, D], f16, kind="ExternalInput")
    wo_d = nc.dram_tensor("WoT", [D, D], f16, kind="ExternalInput")
    eb_d = nc.dram_tensor("expBT", [HEADS, n, n], f16, kind="ExternalInput")
    out_d = nc.dram_tensor("out", [bpc, n, D], f16, kind="ExternalOutput")

    with tile.TileContext(nc) as tc:
        pers = tc.alloc_tile_pool(name="pers", bufs=1)
        # ---- persistent tiles
        QT = [[pers.tile([128, n], f16, tag=f"qt{bi}_{ip}", name=f"qt{bi}_{ip}")
               for ip in range(KP)] for bi in range(bpc)]
        KT = [[pers.tile([128, n], f16, tag=f"kt{bi}_{ip}", name=f"kt{bi}_{ip}")
               for ip in range(KP)] for bi in range(bpc)]
        VA = [[pers.tile([128, HEADS * (DH + 1)], f16, tag=f"va{bi}_{nt}",
                         name=f"va{bi}_{nt}") for nt in range(NT)]
              for bi in range(bpc)]
        AO = [[pers.tile([128, n], f16, tag=f"ao{bi}_{kp}", name=f"ao{bi}_{kp}")
               for kp in range(KP)] for bi in range(bpc)]
        # each weight matrix lives in one [128, KP, 512] tile (one DMA each)
        w4 = {}
        for wname, wd in (("q", wq_d), ("k", wk_d), ("v", wv_d), ("o", wo_d)):
            t = pers.tile([128, KP, D], f16, tag=f"w{wname}", name=f"w{wname}")
            w4[wname] = t
        xtp = tc.alloc_tile_pool(name="xt", bufs=1)
        xt = [xtp.tile([128, KP, n], f16, tag=f"x{bi}", name=f"x{bi}")
              for bi in range(bpc)]

        ebp = tc.alloc_tile_pool(name="eb", bufs=eb_bufs)
        etp = tc.alloc_tile_pool(name="et", bufs=et_bufs)
        lrp = tc.alloc_tile_pool(name="lr", bufs=lb_bufs)
        fop = tc.alloc_tile_pool(name="fop", bufs=1)
        ps = tc.alloc_tile_pool(name="ps", bufs=1, space="PSUM")

        # ---- prologue DMAs (one chained DMA per tensor)
        def dma_w(wname, wd):
            dst = w4[wname][:]
            src = wd.rearrange("(kp p) c -> p kp c", p=128)
            nc.sync.dma_start(out=dst, in_=src)

        def dma_x(bi, split=False):
            if split:
                for kp in range(KP):
                    nc.sync.dma_start(
                        out=xt[bi][:, kp, :],
                        in_=xT_d[bi, kp * 128:(kp + 1) * 128, :])
            else:
                nc.sync.dma_start(
                    out=xt[bi][:], in_=xT_d[bi].rearrange("(kp p) j -> p kp j", p=128))

        pool_jts = set(range(3, 3 + pool_mul_mod))
        av_jt_order = [j for j in range(NT) if j not in pool_jts] + sorted(pool_jts)
        eb_tiles = {}

        def emit_eb_dma(h, jp):
            """Load jt pair (2*jp, 2*jp+1) of head h as one [128, 2, n] tile."""
            t = ebp.tile([128, 2, n], f16, tag="eb", name="eb")
            nc.sync.dma_start(
                out=t[:],
                in_=eb_d[h, 2 * jp * 128:(2 * jp + 2) * 128, :].rearrange(
                    "(two p) i -> p two i", two=2))
            eb_tiles[(h, 2 * jp)] = t[:, 0, :]
            eb_tiles[(h, 2 * jp + 1)] = t[:, 1, :]

        dma_w("q", wq_d)
        dma_x(0, split=bool(x0_split))
        dma_w("k", wk_d)
        dma_x(1)
        dma_w("v", wv_d)
        dma_w("o", wo_d)
        for jp in range(NT // 2):
            emit_eb_dma(0, jp)

        # ---- PE warm-up: keep the tensor engine continuously busy through
        # the prologue DMAs so its p-state clock is fully ramped (213ns/row
        # instead of 427+) when the first real projection matmuls arrive.
        if warmup_mms:
            scr = pers.tile([128, 512], f16, tag="scr", name="scr")
            nc.gpsimd.memset(scr[:], 0.0)
            wps = ps.tile([128, 512], f32, tag="mm", bufs=sim_bufs, name="wps")
            for _ in range(warmup_mms):
                nc.tensor.matmul(wps[:], scr[:, 0:128], scr[:],
                                 start=True, stop=True)

        # ---- emitters -----------------------------------------------------
        inj_ring = [0]

        def inj_tag():
            return "ot"

        def emit_qk_chunk(wname, DST, bi, ip, ring=None, halves=(0, 1), first_split=False):
            W_s = w4[wname]
            wide = len(halves) == 2 and not split_inj
            pt = ps.tile([128, 1024 if wide else 512], f32,
                         tag=ring or inj_tag(), bufs=ot_bufs, name="pt")
            for i, nh in enumerate(halves):
                base = i * 512 if wide else 0
                for kp in range(KP):
                    nc.tensor.matmul(
                        pt[:, base:base + 512],
                        W_s[:, kp, ip * 128:(ip + 1) * 128],
                        xt[bi][:, kp, nh * 512:(nh + 1) * 512],
                        start=(kp == 0), stop=(kp == KP - 1),
                    )
                if not wide:
                    nc.vector.tensor_copy(
                        out=DST[bi][ip][:, nh * 512:(nh + 1) * 512], in_=pt[:])
            if wide:
                if first_split:
                    # sim jt0 needs only K cols 0:128 / Q cols 0:512 - copy
                    # those first so the first sim fires sooner.
                    c0 = 128 if wname == "k" else 512
                    nc.vector.tensor_copy(out=DST[bi][ip][:, 0:c0], in_=pt[:, 0:c0])
                    nc.vector.tensor_copy(out=DST[bi][ip][:, c0:], in_=pt[:, c0:])
                else:
                    nc.vector.tensor_copy(out=DST[bi][ip][:], in_=pt[:])

        def emit_v_chunk(bi, t2):
            pt = ps.tile([128, 1024], f32, tag=inj_tag(), bufs=ot_bufs, name="pt")
            for b in range(2):
                nt = 2 * t2 + b
                for kp in range(KP):
                    nc.tensor.matmul(
                        pt[:, b * 512:(b + 1) * 512],
                        xt[bi][:, kp, nt * 128:(nt + 1) * 128],
                        w4["v"][:, kp, :],
                        start=(kp == 0), stop=(kp == KP - 1),
                    )
            for b in range(2):
                nt = 2 * t2 + b
                va = VA[bi][nt]
                nc.gpsimd.memset(va[:], 1.0)
                dst3 = va[:].rearrange("p (h c) -> p h c", c=DH + 1)[:, :, 0:DH]
                src3 = pt[:, b * 512:(b + 1) * 512].rearrange("p (h c) -> p h c", c=DH)
                nc.vector.tensor_copy(out=dst3, in_=src3)

        def emit_f_chunk(bi, t2, ring=None):
            pt = ps.tile([128, 1024], f32, tag=ring or inj_tag(), bufs=ot_bufs,
                         name="pt")
            for b in range(2):
                nt = 2 * t2 + b
                for kp in range(KP):
                    nc.tensor.matmul(
                        pt[:, b * 512:(b + 1) * 512],
                        AO[bi][kp][:, nt * 128:(nt + 1) * 128],
                        w4["o"][:, kp, :],
                        start=(kp == 0), stop=(kp == KP - 1),
                    )
            for b in range(2):
                nt = 2 * t2 + b
                fo = fop.tile([128, 512], f16, tag="fo", bufs=fo_bufs, name="fo")
                nc.scalar.activation(fo[:], pt[:, b * 512:(b + 1) * 512], Copy)
                nc.sync.dma_start(out=out_d[bi, nt * 128:(nt + 1) * 128, :], in_=fo[:])

        pend = {"av": None}

        def start_av(h, bi, et_row):
            ot = ps.tile([DH + 1, 1024], f32, tag="ot", bufs=ot_bufs, name="ot")
            mms = [(ihh, jt) for ihh in range(2) for jt in av_jt_order]
            pend["av"] = {"h": h, "bi": bi, "et": et_row, "ot": ot, "mms": mms}

        def step_av(nmm):
            st = pend["av"]
            if st is None:
                return
            h, bi, et_row, ot = st["h"], st["bi"], st["et"], st["ot"]
            while nmm > 0 and st["mms"]:
                ihh, jt = st["mms"].pop(0)
                nc.tensor.matmul(
                    ot[:, ihh * 512:(ihh + 1) * 512],
                    VA[bi][jt][:, h * (DH + 1):(h + 1) * (DH + 1)],
                    et_row[jt][:, ihh * 512:(ihh + 1) * 512],
                    start=(jt == av_jt_order[0]), stop=(jt == av_jt_order[-1]),
                )
                nmm -= 1
            if not st["mms"]:
                emit_norm(h, bi, ot)
                pend["av"] = None

        def emit_norm(h, bi, ot):
            # reciprocal straight off the psum l-row into row 64 of the
            # broadcast tile, gpsimd broadcast from partition 64.
            lb = lrp.tile([DH + 1, n], f32, tag="lb", name="lb")
            nc.vector.reciprocal_approx_fast(out=lb[DH:DH + 1, :], in_=ot[DH:DH + 1, :])
            nc.gpsimd.partition_broadcast(lb[0:DH, :], lb[DH:DH + 1, :], channels=DH)
            if h % 2 == 0:
                nc.vector.tensor_mul(
                    out=AO[bi][h // 2][0:DH, :], in0=ot[0:DH, :], in1=lb[0:DH, :])
            else:
                tmpo = lrp.tile([DH, n], f16, tag="tmpo", bufs=tmpo_bufs, name="tmpo")
                nc.vector.tensor_mul(out=tmpo[:], in0=ot[0:DH, :], in1=lb[0:DH, :])
                nc.sync.dma_start(out=AO[bi][h // 2][DH:128, :], in_=tmpo[:])

        # ---- prologue compute: QK projections for ip=0 (heads 0,1).
        # The sim ring is empty this early, so borrow it: four chunks then
        # rotate through four psum slots instead of two.
        for bi in range(bpc):
            emit_qk_chunk("q", QT, bi, 0, ring="mm" if proq_mm else None,
                          first_split=(bi == 0 and bool(head_split)))
            emit_qk_chunk("k", KT, bi, 0, ring="mm" if proq_mm else None,
                          first_split=(bi == 0 and bool(head_split)))

        # ---- deadline-scheduled inject: chunk -> emission slot
        sched_ip1 = list(sched_ip1)
        sched = {}
        slots_v0 = [0, 2, 4, 6]
        slots_v1 = [8, 10, 12, 14]
        slots_ip = {1: sched_ip1, 2: [36, 44, 52, 60], 3: [68, 76, 84, 92]}
        if split_inj:
            for t2 in range(T2):
                sched[slots_v0[t2]] = [("vh", 0, t2, 0), ("vh", 0, t2, 1)]
                sched[slots_v1[t2]] = [("vh", 1, t2, 0), ("vh", 1, t2, 1)]
            for ip in range(1, KP):
                for i, (kind, bi) in enumerate((("q", 0), ("k", 0), ("q", 1), ("k", 1))):
                    sched[slots_ip[ip][i]] = [(kind + "h", bi, ip, 0),
                                              (kind + "h", bi, ip, 1)]
        else:
            for t2 in range(T2):
                sched[slots_v0[t2]] = [("v", 0, t2)]
                sched[slots_v1[t2]] = [("v", 1, t2)]
            for ip in range(1, KP):
                for i, (kind, bi) in enumerate((("q", 0), ("k", 0), ("q", 1), ("k", 1))):
                    sched[slots_ip[ip][i]] = [(kind, bi, ip)]
        for t2 in range(T2):
            sched[120 + 2 * t2] = [("f", 0, t2)]

        def emit_chunk(c):
            kind = c[0]
            if kind == "v":
                emit_v_chunk(c[1], c[2])
            elif kind == "vh":
                emit_v_chunk(c[1], c[2], blocks=(c[3],))
            elif kind == "q":
                emit_qk_chunk("q", QT, c[1], c[2])
            elif kind == "k":
                emit_qk_chunk("k", KT, c[1], c[2])
            elif kind == "qh":
                emit_qk_chunk("q", QT, c[1], c[2], halves=(c[3],))
            elif kind == "kh":
                emit_qk_chunk("k", KT, c[1], c[2], halves=(c[3],))
            elif kind == "f":
                emit_f_chunk(c[1], c[2])

        # ---- main attention loop; last head pair runs h-odd first so the
        # final stream is an even head (its norm-mul writes AO directly,
        # shortening the tail by one SBUF->SBUF DMA hop).
        stream_order = {}
        for hp in range(HP):
            h0, h1 = 2 * hp, 2 * hp + 1
            if hp == HP - 1:
                stream_order[hp] = [(h1, 0), (h1, 1), (h0, 0), (h0, 1)]
            else:
                stream_order[hp] = [(h0, 0), (h0, 1), (h1, 0), (h1, 1)]

        slot = 0
        for hp in range(HP):
            streams = stream_order[hp]
            first_h = streams[0][0]
            other_h = streams[2][0]
            for sidx, (h, bi) in enumerate(streams):
                is_last = (hp == HP - 1 and sidx == 3)
                ot_last = None
                if is_last:
                    ot_last = ps.tile([DH + 1, 1024], f32, tag="ot",
                                      bufs=ot_bufs, name="ot_last")
                et_row = []
                sp_hoist = [None]
                for jt in range(NT):
                    # eb prefetch (jt pairs): sidx1 fetches this pair's other
                    # head, sidx2 fetches the next head-pair's first head.
                    if jt % 2 == 0:
                        if sidx == 1:
                            emit_eb_dma(other_h, jt // 2)
                        elif sidx == 2 and hp + 1 < HP:
                            emit_eb_dma(stream_order[hp + 1][0][0], jt // 2)
                    po = (h % 2) * 64

                    def emit_sim(j):
                        spj = ps.tile([128, 1024], f32, tag="mm", bufs=sim_bufs,
                                      name="spj")
                        for ihh in range(2):
                            nc.tensor.matmul(
                                spj[:, ihh * 512:(ihh + 1) * 512],
                                KT[bi][hp][po:po + 64, j * 128:(j + 1) * 128],
                                QT[bi][hp][po:po + 64, ihh * 512:(ihh + 1) * 512],
                                start=True, stop=True,
                            )
                        return spj

                    if jt == 1 and sp_hoist[0] is not None:
                        sp = sp_hoist[0]
                    else:
                        sp = emit_sim(jt)
                    if hoist_sim and jt == 0 and pend["av"] is not None:
                        sp_hoist[0] = emit_sim(1)
                    eq = etp.tile([128, n], f16, tag="eq", bufs=8, name="eq")
                    nc.scalar.activation(eq[:], sp[:], Exp)
                    et = etp.tile([128, n], f16, tag="et", name="et")
                    use_pool = (not is_last) and jt in pool_jts
                    mul_eng = nc.gpsimd if use_pool else nc.vector
                    mul_eng.tensor_mul(out=et[:], in0=eq[:], in1=eb_tiles[(h, jt)])
                    et_row.append(et)
                    if is_last:
                        for ihh in range(2):
                            nc.tensor.matmul(
                                ot_last[:, ihh * 512:(ihh + 1) * 512],
                                VA[bi][jt][:, h * (DH + 1):(h + 1) * (DH + 1)],
                                et[:, ihh * 512:(ihh + 1) * 512],
                                start=(jt == 0), stop=(jt == NT - 1),
                            )
                    if jt == av_flush_slot:
                        step_av(99)
                    if slot in sched:
                        for c in sched.pop(slot):
                            emit_chunk(c)
                    slot += 1
                if is_last:
                    emit_norm(h, bi, ot_last)
                else:
                    emit_av(h, bi, et_row)
        step_av(99)
        # drain: anything left, then F for batch 1
        for s in sorted(sched):
            for c in sched[s]:
                emit_chunk(c)
        for t2 in range(T2):
            emit_f_chunk(1, t2, ring=("mm" if (f_drain_mm and t2 % 2) else None))

        for p in (ps, fop, lrp, etp, ebp, xtp, pers):
            p.release()

    nc.compile()
    return nc


def prep_inputs(x, Wq, Wk, Wv, rel_bias, Wo, bo, n=N, bpc=BPC, ncores=NCORES):
    """Host-side sharding/layout prep. Returns in_maps (one dict per core)."""
    f16 = np.float16
    x = np.asarray(x, dtype=np.float32)
    xT = np.ascontiguousarray(x.transpose(0, 2, 1)).astype(f16)   # [B, D, n]
    WqT = np.ascontiguousarray(Wq.T * np.float32(SCALE)).astype(f16)
    WkT = np.ascontiguousarray(Wk.T).astype(f16)
    WvT = np.ascontiguousarray(Wv.T).astype(f16)
    WoT = np.ascontiguousarray(Wo.T).astype(f16)
    expBT = np.ascontiguousarray(
        np.exp(np.asarray(rel_bias, dtype=np.float32).transpose(0, 2, 1))
    ).astype(f16)                                                  # [H, n(j), n(i)]
    in_maps = []
    for c in range(ncores):
        in_maps.append({
            "xT": np.ascontiguousarray(xT[c * bpc:(c + 1) * bpc]),
            "WqT": WqT, "WkT": WkT, "WvT": WvT, "WoT": WoT,
            "expBT": expBT,
        })
    return in_maps


_CACHE = {}


def kernel(x, Wq, Wk, Wv, rel_bias, Wo, bo):
    from concourse.bass_utils import run_bass_kernel_spmd

    if "nc" not in _CACHE:
        _CACHE["nc"] = build_nc()
    nc = _CACHE["nc"]
    in_maps = prep_inputs(x, Wq, Wk, Wv, rel_bias, Wo, bo)
    res = run_bass_kernel_spmd(nc, in_maps, core_ids=list(range(NCORES)))
    out = np.concatenate([res.results[c]["out"] for c in range(NCORES)], axis=0)
    out = out + np.asarray(bo, dtype=np.float32)[None, None, :]
    return np.ascontiguousarray(out, dtype=np.float32)
